# revision 1
# baseline (speedup 1.0000x reference)
"""Trainium2 Bass kernel for nn_Model1 (lag-weighted long-run covariance + MLP).

Math: the 129-lag weighted covariance collapses algebraically:
    sum_l w_l * (Xc @ Y_l.T) = Xc @ (sum_l w_l Y_l).T
where Y_l is the (masked) l-shift of Xc.  So cov = (Xc @ P.T + N @ Xc.T)/d
with P, N two 65-tap causal FIR filters of Xc.  Centering is pushed through
the filters as rank-1 corrections so all GEMMs run on UNCENTERED X:
    cov.T * d = U.T - m (x) alpha - beta (x) m
with U = X@P0.T + N0@X.T (P0,N0 = filters of raw X), m = row means, and
alpha/beta derived from 5 reduction vectors that ride along as extra rhs
columns of the U matmuls.

Distribution (8 cores):
  - cov stage: shard time axis (512 cols/core), one AllReduce of [256,261] f32
  - MLP: tensor-parallel over hidden (512/core), AllGather (bf16) between
    fc1->fc2 and fc2->fc3; fc3 emits batch-major so proj shards the output
    columns; final gather is a host-side concat.
All heavy GEMMs use bf16 operands with fp32 PSUM accumulation.
"""
import math
import numpy as np
import ml_dtypes

NCORES = 8
Q = 64
NN = 256          # n (batch/rows of X)
DD = 4096         # d (time axis)
HID = 4096
Y0 = 512
HSH = HID // NCORES    # 512 hidden shard per core
NBLK = DD // 128       # 32 time blocks
BPC = NBLK // NCORES   # 4 blocks per core
KB1 = (DD + DD) // 128  # 64 fc1 contraction blocks
KB = HID // 128         # 32 fc2/fc3 contraction blocks
XHS = 264              # xh per-block col stride (256 data + 1 ones + pad)

bf16 = ml_dtypes.bfloat16

_CACHE = {}


# ----------------------------------------------------------------------------
# host-side weight-only precompute
# ----------------------------------------------------------------------------
def _erf(x):
    # vectorized erf via math.erf (no scipy dependency)
    return np.vectorize(math.erf, otypes=[np.float64])(x)


def _gelu64(x):
    return 0.5 * x * (1.0 + _erf(x / np.sqrt(2.0)))


def _filters(inputs):
    f64 = lambda k: np.asarray(inputs[k], np.float64)
    lags = np.arange(-Q, Q + 1, dtype=np.float64)[:, None]
    h = _gelu64(lags @ f64("wn_w1") + f64("wn_b1"))
    w = (h @ f64("wn_w2") + f64("wn_b2"))[:, 0]
    wp = w[Q:]                                    # l = 0..Q
    wnv = np.concatenate([[0.0], w[:Q][::-1]])    # wnv[l] = w[Q-l], l=1..Q
    v = np.arange(128)[:, None]
    u = np.arange(128)[None, :]
    dvu = v - u
    d2 = dvu + 128
    A0 = np.where((dvu >= 0) & (dvu <= Q), wp[np.clip(dvu, 0, Q)], 0.0)
    A1 = np.where((d2 >= 0) & (d2 <= Q), wp[np.clip(d2, 0, Q)], 0.0)
    B0 = np.where((dvu >= 1) & (dvu <= Q), wnv[np.clip(dvu, 0, Q)], 0.0)
    B1 = np.where((d2 >= 1) & (d2 <= Q), wnv[np.clip(d2, 0, Q)], 0.0)
    t = np.arange(DD)
    lim = np.minimum(Q, DD - 1 - t)
    g_p = np.cumsum(wp)[lim]
    g_n = np.cumsum(wnv)[lim]
    gamma = float(g_p.sum() + g_n.sum())
    return A0, A1, B0, B1, g_p, g_n, gamma


# ----------------------------------------------------------------------------
# bass program
# ----------------------------------------------------------------------------
def build(gamma: float, use_gelu=True):
    import concourse.bacc as bacc
    import concourse.tile as tile
    import concourse.mybir as mybir

    dt32 = mybir.dt.float32
    dt16 = mybir.dt.bfloat16
    GELU = (mybir.ActivationFunctionType.Gelu if use_gelu
            else mybir.ActivationFunctionType.Identity)
    COPY = mybir.ActivationFunctionType.Copy

    nc = bacc.Bacc("TRN2", target_bir_lowering=False, debug=False,
                   num_devices=NCORES)
    mm = nc.tensor.matmul
    RG = [list(range(NCORES))]

    # ---- I/O ----
    xt_d = nc.dram_tensor("xt", [128, NBLK * 256], dt16, kind="ExternalInput").ap()
    xh_d = nc.dram_tensor("xh", [128, 5 * XHS], dt16, kind="ExternalInput").ap()
    tp_d = nc.dram_tensor("tp", [128, 512], dt16, kind="ExternalInput").ap()
    aux_d = nc.dram_tensor("aux", [128, BPC * 3], dt16, kind="ExternalInput").ap()
    w1_d = nc.dram_tensor("w1", [128, KB1 * 512], dt16, kind="ExternalInput").ap()
    w2_d = nc.dram_tensor("w2", [128, KB * 512], dt16, kind="ExternalInput").ap()
    w3_d = nc.dram_tensor("w3", [128, KB * 512], dt16, kind="ExternalInput").ap()
    w2l_d = nc.dram_tensor("w2l", [128, 4 * 512], dt16, kind="ExternalInput").ap()
    w3l_d = nc.dram_tensor("w3l", [128, 4 * 512], dt16, kind="ExternalInput").ap()
    pj_d = nc.dram_tensor("pj", [128, 2 * 512], dt16, kind="ExternalInput").ap()
    b1_d = nc.dram_tensor("b1", [128, 4], dt32, kind="ExternalInput").ap()
    b2_d = nc.dram_tensor("b2", [128, 4], dt32, kind="ExternalInput").ap()
    b3_d = nc.dram_tensor("b3", [1, 512], dt16, kind="ExternalInput").ap()
    out_d = nc.dram_tensor("out", [Y0, HSH], dt32, kind="ExternalOutput").ap()

    with tile.TileContext(nc) as tc:
        with (
            tc.tile_pool(name="cst", bufs=1) as cst,
            tc.tile_pool(name="pn", bufs=3) as pnp,
            tc.tile_pool(name="wst", bufs=8) as wst,
            tc.tile_pool(name="wsx", bufs=8) as wsx,
            tc.tile_pool(name="psA", bufs=1, space="PSUM") as psA,
            tc.tile_pool(name="psB", bufs=2, space="PSUM") as psB,
            tc.tile_pool(name="dram", bufs=1, space="DRAM") as drp,
        ):
            # ---------- persistent SBUF (latency-critical first) ----------
            xh_t = cst.tile([128, 5 * XHS], dt16, tag="xh")
            nc.sync.dma_start(xh_t, xh_d)
            tp_t = cst.tile([128, 512], dt16, tag="tp")
            nc.sync.dma_start(tp_t, tp_d)
            aux_t = cst.tile([128, BPC * 3], dt16, tag="aux")
            nc.sync.dma_start(aux_t, aux_d)
            b1_t = cst.tile([128, 4], dt32, tag="b1")
            nc.sync.dma_start(b1_t, b1_d)
            b2_t = cst.tile([128, 4], dt32, tag="b2")
            nc.sync.dma_start(b2_t, b2_d)
            b3_t = cst.tile([1, 512], dt16, tag="b3")
            nc.sync.dma_start(b3_t, b3_d)
            pj_t = cst.tile([128, 2 * 512], dt16, tag="pj")
            nc.sync.dma_start(pj_t, pj_d)
            w2l_t = cst.tile([128, 4 * 512], dt16, tag="w2l")
            nc.sync.dma_start(w2l_t, w2l_d)
            w3l_t = cst.tile([128, 4 * 512], dt16, tag="w3l")
            nc.sync.dma_start(w3l_t, w3l_d)
            ones_t = cst.tile([128, 1], dt16, tag="ones")
            nc.vector.memset(ones_t, 1.0)
            onesr_t = cst.tile([1, 128], dt16, tag="onesr")
            nc.vector.memset(onesr_t, 1.0)
            xt_t = cst.tile([128, NBLK * 256], dt16, tag="xt")
            for sp in range(4):
                w = NBLK * 256 // 4
                nc.sync.dma_start(xt_t[:, w * sp: w * (sp + 1)],
                                  xt_d[:, w * sp: w * (sp + 1)])

            # bounce buffers (DRAM)
            arA_i = drp.tile([261, NN], dt32, tag="arA_i")
            arA_o = drp.tile([261, NN], dt32, tag="arA_o", addr_space="Shared")
            g1_i = drp.tile([HSH, NN], dt16, tag="g1_i")
            g1_o = drp.tile([HID, NN], dt16, tag="g1_o", addr_space="Shared")
            g2_i = drp.tile([HSH, NN], dt16, tag="g2_i")
            g2_o = drp.tile([HID, NN], dt16, tag="g2_o", addr_space="Shared")

            # ---------- stage 1: cov partials over local time blocks ----------
            # u_ps[ic]: UT chunk [a in 128*ic.., b] ; vec_ps rows 0:3 = (r,a,c),
            # row 32 = p, row 64 = q  (separate partition groups, one bank)
            u_ps = [psA.tile([128, 257], dt32, tag=f"acc{ic}", name=f"u{ic}")
                    for ic in range(2)]
            rac_ps = psA.tile([3, 256], dt32, tag="acc2", name="rac_ps")
            q_ps = psA.tile([1, 256], dt32, tag="acc3", name="q_ps")
            for bl in range(BPC):
                xb = xh_t[:, XHS * bl: XHS * bl + 256]
                xb1 = xh_t[:, XHS * (bl + 1): XHS * (bl + 1) + 256]
                pt_ps = psB.tile([128, 256], dt32, tag="rot", name="pt_ps")
                mm(pt_ps, tp_t[:, 0:128], xb, start=True, stop=False)
                mm(pt_ps, tp_t[:, 128:256], xb1, start=False, stop=True)
                pt_sb = pnp.tile([128, 256], dt16, tag="ptsb", name="pt_sb")
                nc.vector.tensor_copy(pt_sb, pt_ps)
                nt_ps = psB.tile([128, 256], dt32, tag="rot", name="nt_ps")
                mm(nt_ps, tp_t[:, 256:384], xb, start=True, stop=False)
                mm(nt_ps, tp_t[:, 384:512], xb1, start=False, stop=True)
                nt_sb = pnp.tile([128, 256], dt16, tag="ntsb", name="nt_sb")
                nc.vector.tensor_copy(nt_sb, nt_ps)
                first, last = bl == 0, bl == BPC - 1
                xbo = xh_t[:, XHS * bl: XHS * bl + 257]   # + ones col -> p
                for ic in range(2):
                    xbc = xh_t[:, XHS * bl + 128 * ic: XHS * bl + 128 * ic + 128]
                    mm(u_ps[ic][:, 0:257], pt_sb[:, 128 * ic:128 * ic + 128], xbo,
                       start=first, stop=False)
                    mm(u_ps[ic][:, 0:256], xbc, nt_sb, start=False, stop=last)
                # reduction vectors directly as psum ROWS
                mm(rac_ps, aux_t[:, 3 * bl:3 * bl + 3], xb,
                   start=first, stop=last)
                mm(q_ps, ones_t, nt_sb, start=first, stop=last)

            # pack to bounce + AllReduce  (arA layout: rows 0:256 = UT,
            # 256:259 = r/a/c, 259 = p, 260 = q)
            stgs = []
            for ic in range(2):
                stg = cst.tile([128, 257], dt32, tag=f"stg{ic}", name=f"stg{ic}")
                nc.vector.tensor_scalar_mul(stg, u_ps[ic], 1.0 / DD)
                nc.scalar.dma_start(arA_i[128 * ic:128 * ic + 128, :],
                                    stg[:, 0:256])
                stgs.append(stg)
            stv_rac = cst.tile([3, 256], dt32, tag="stv_rac", name="stv_rac")
            nc.vector.tensor_copy(stv_rac, rac_ps)
            stv_q = cst.tile([1, 256], dt32, tag="stv_q", name="stv_q")
            nc.vector.tensor_copy(stv_q, q_ps)
            nc.scalar.dma_start(arA_i[256:259, :], stv_rac)
            nc.scalar.dma_start(arA_i[259:260, 0:128], stgs[0][:, 256:257])
            nc.scalar.dma_start(arA_i[259:260, 128:256], stgs[1][:, 256:257])
            nc.scalar.dma_start(arA_i[260:261, :], stv_q)
            nc.gpsimd.collective_compute(
                "AllReduce", mybir.AluOpType.add, replica_groups=RG,
                ins=[arA_i.opt()], outs=[arA_o.opt()])

            # ---------- stage 2b: fc1 X-half (overlaps AllReduce) ----------
            f1_ps = [psA.tile([128, 256], dt32, tag=f"acc{hh}", name=f"f1_{hh}")
                     for hh in range(4)]
            for k2 in range(NBLK // 2):
                wt = wsx.tile([128, 1024], dt16, tag="wx", name="wt")
                nc.sync.dma_start(wt, w1_d[:, 1024 * k2: 1024 * k2 + 1024])
                for dk in range(2):
                    k = 2 * k2 + dk
                    for hh in range(4):
                        mm(f1_ps[hh],
                           wt[:, 512 * dk + 128 * hh: 512 * dk + 128 * hh + 128],
                           xt_t[:, 256 * k:256 * k + 256],
                           start=(k == 0), stop=False)

            # ---------- stage 2: G^T = X @ W1c  (cov-independent, in AR shadow)
            g_ps = [psA.tile([128, 512], dt32, tag=f"acc{4 + ib}", name=f"g_{ib}")
                    for ib in range(2)]
            for k2 in range(NBLK // 2):
                wt = wst.tile([128, 1024], dt16, tag="wc", name="wt")
                nc.sync.dma_start(
                    wt, w1_d[:, 1024 * (NBLK // 2 + k2): 1024 * (NBLK // 2 + k2) + 1024])
                for dk in range(2):
                    k = 2 * k2 + dk
                    for ib in range(2):
                        mm(g_ps[ib],
                           xt_t[:, 256 * k + 128 * ib: 256 * k + 128 * ib + 128],
                           wt[:, 512 * dk: 512 * dk + 512],
                           start=(k == 0), stop=(k == NBLK - 1))
            gT = cst.tile([128, 2 * 512], dt16, tag="gT")
            for ib in range(2):
                nc.vector.tensor_copy(gT[:, 512 * ib:512 * ib + 512], g_ps[ib])


            # ---------- stage 3: corrections + covT ----------
            ured = [cst.tile([128, 256], dt32, tag=f"stg{ic}", name=f"ured{ic}")
                    for ic in range(2)]
            for ic in range(2):
                nc.scalar.dma_start(ured[ic], arA_o[128 * ic:128 * ic + 128, :])
            rows32 = {}
            for i, nm in enumerate(("r", "a", "c", "p", "q")):
                rw = cst.tile([1, NN], dt32, tag=f"row_{nm}", name=f"row_{nm}")
                nc.scalar.dma_start(rw, arA_o[256 + i:257 + i, :])
                rows32[nm] = rw
            # staged U (and its p column) are pre-divided by D, so alpha and
            # beta are divided by D here too; m stays un-divided for the
            # rank-1 products.
            m32 = cst.tile([1, NN], dt32, tag="m32")
            nc.vector.tensor_scalar_mul(m32, rows32["r"], 1.0 / DD)
            al32 = cst.tile([1, NN], dt32, tag="al32")
            nc.vector.tensor_add(al32, rows32["a"], rows32["q"])
            nc.vector.tensor_scalar_mul(al32, al32, 1.0 / DD)
            gm32 = cst.tile([1, NN], dt32, tag="gm32")
            nc.vector.tensor_scalar_mul(gm32, m32, gamma / DD)
            nc.vector.tensor_sub(al32, al32, gm32)
            be32 = cst.tile([1, NN], dt32, tag="be32")
            nc.vector.tensor_scalar_mul(be32, rows32["c"], 1.0 / DD)
            nc.vector.tensor_add(be32, rows32["p"], be32)
            m16 = cst.tile([1, NN], dt16, tag="m16")
            nc.vector.tensor_copy(m16, m32)
            al16 = cst.tile([1, NN], dt16, tag="al16")
            nc.vector.tensor_copy(al16, al32)
            be16 = cst.tile([1, NN], dt16, tag="be16")
            nc.vector.tensor_copy(be16, be32)

            covt = cst.tile([128, 2 * 256], dt16, tag="covt")
            for ic in range(2):
                corr = psB.tile([128, 256], dt32, tag="rot", name="corr")
                mm(corr, m16[:, 128 * ic:128 * ic + 128], al16,
                   start=True, stop=False)
                mm(corr, be16[:, 128 * ic:128 * ic + 128], m16,
                   start=False, stop=True)
                nc.vector.tensor_sub(covt[:, 256 * ic:256 * ic + 256],
                                     ured[ic], corr)

            # ---------- stage 5: fc1 cov contribution = G @ M + gelu ----------
            for hh in range(4):
                for ib in range(2):
                    mm(f1_ps[hh], gT[:, 512 * ib + 128 * hh: 512 * ib + 128 * hh + 128],
                       covt[:, 256 * ib:256 * ib + 256],
                       start=False, stop=(ib == 1))
            a1loc = cst.tile([128, 4 * 256], dt16, tag="a1loc")
            for hh in range(4):
                nc.scalar.activation(a1loc[:, 256 * hh:256 * hh + 256],
                                     f1_ps[hh], GELU, bias=b1_t[:, hh:hh + 1])
                nc.scalar.dma_start(g1_i[128 * hh:128 * hh + 128, :],
                                    a1loc[:, 256 * hh:256 * hh + 256])

            # ---------- stage 6: AllGather a1 ----------
            # fc2/fc3 weights fully resident; dispatched on the scalar stream
            # after gelu (= after the AllReduce) so the SDMA engines stay quiet
            # during the collective; they stream during AllGather-1.
            w2R = cst.tile([128, KB * 512], dt16, tag="w2R")
            for sp in range(16):
                w = KB * 512 // 16
                nc.scalar.dma_start(w2R[:, w * sp: w * (sp + 1)],
                                    w2_d[:, w * sp: w * (sp + 1)])
            w3R = cst.tile([128, KB * 512], dt16, tag="w3R", name="w3R")
            for sp in range(16):
                w = KB * 512 // 16
                nc.scalar.dma_start(w3R[:, w * sp: w * (sp + 1)],
                                    w3_d[:, w * sp: w * (sp + 1)])
            nc.gpsimd.collective_compute(
                "AllGather", mybir.AluOpType.bypass, replica_groups=RG,
                ins=[g1_i.opt()], outs=[g1_o.opt()])
            a1f = cst.tile([128, NBLK * 256], dt16, tag="a1f")
            for b in range(NBLK):
                eng = nc.sync if b % 2 == 0 else nc.gpsimd
                eng.dma_start(a1f[:, 256 * b:256 * b + 256],
                              g1_o[128 * b:128 * b + 128, :])

            # ---------- stage 7: fc2 + gelu ----------
            f2_ps = [psA.tile([128, 256], dt32, tag=f"acc{hh}", name=f"f2_{hh}")
                     for hh in range(4)]
            # local chunk contribution from a1loc (runs during AllGather-1);
            # the corresponding k-blocks of w2R are zeroed host-side
            for j in range(BPC):
                for hh in range(4):
                    mm(f2_ps[hh],
                       w2l_t[:, 512 * j + 128 * hh: 512 * j + 128 * hh + 128],
                       a1loc[:, 256 * j:256 * j + 256],
                       start=(j == 0), stop=False)
            for k in range(KB):
                for hh in range(4):
                    mm(f2_ps[hh],
                       w2R[:, 512 * k + 128 * hh: 512 * k + 128 * hh + 128],
                       a1f[:, 256 * k:256 * k + 256],
                       start=False, stop=(k == KB - 1))
            a2loc = cst.tile([128, 4 * 256], dt16, tag="a2loc")
            for hh in range(4):
                nc.scalar.activation(a2loc[:, 256 * hh:256 * hh + 256],
                                     f2_ps[hh], GELU, bias=b2_t[:, hh:hh + 1])
                nc.scalar.dma_start(g2_i[128 * hh:128 * hh + 128, :],
                                    a2loc[:, 256 * hh:256 * hh + 256])

            # ---------- stage 8: AllGather a2 ----------
            nc.gpsimd.collective_compute(
                "AllGather", mybir.AluOpType.bypass, replica_groups=RG,
                ins=[g2_i.opt()], outs=[g2_o.opt()])
            a2f = cst.tile([128, NBLK * 256], dt16, tag="a2f")
            for b in range(NBLK):
                eng = nc.sync if b % 2 == 0 else nc.gpsimd
                eng.dma_start(a2f[:, 256 * b:256 * b + 256],
                              g2_o[128 * b:128 * b + 128, :])

            # ---------- stage 9: fc3 (batch-major out) ----------
            f3_ps = [psA.tile([128, 512], dt32, tag=f"acc{ii}", name=f"f3_{ii}")
                     for ii in range(2)]
            o3_t = cst.tile([128, 2 * 512], dt16, tag="o3")
            for ii in range(2):
                for j in range(BPC):   # local chunk, runs during AllGather-2
                    mm(f3_ps[ii],
                       a2loc[:, 256 * j + 128 * ii: 256 * j + 128 * ii + 128],
                       w3l_t[:, 512 * j: 512 * j + 512],
                       start=(j == 0), stop=False)
                for k in range(KB):
                    mm(f3_ps[ii],
                       a2f[:, 256 * k + 128 * ii: 256 * k + 128 * ii + 128],
                       w3R[:, 512 * k: 512 * k + 512],
                       start=False, stop=False)
                mm(f3_ps[ii], onesr_t, b3_t, start=False, stop=True)
                nc.vector.tensor_copy(o3_t[:, 512 * ii:512 * ii + 512], f3_ps[ii])

            # ---------- stage 10: proj ----------
            for pp in range(4):
                po = psB.tile([128, 512], dt32, tag="rot", name="po")
                for ii in range(2):
                    mm(po, pj_t[:, 512 * ii + 128 * pp: 512 * ii + 128 * pp + 128],
                       o3_t[:, 512 * ii:512 * ii + 512],
                       start=(ii == 0), stop=(ii == 1))
                osb = cst.tile([128, 512], dt32, tag=f"osb{pp}", name=f"osb{pp}")
                nc.vector.tensor_copy(osb, po)
                nc.sync.dma_start(out_d[128 * pp:128 * pp + 128, :], osb)

    nc.compile()
    return nc


# ----------------------------------------------------------------------------
# host-side sharding / packing
# ----------------------------------------------------------------------------
def prep_in_maps(inputs):
    X = np.asarray(inputs["X"], np.float32)
    A0, A1, B0, B1, g_p, g_n, gamma = _filters(inputs)

    XT = np.ascontiguousarray(X.T)                      # [D, N]
    xt = XT.reshape(NBLK, 128, NN).transpose(1, 0, 2).reshape(128, NBLK * 256)
    xt = xt.astype(bf16)
    tp = np.concatenate([A0, A1, B0, B1], axis=1).astype(bf16)
    pjT = np.asarray(inputs["proj"], np.float64).T      # [256, 512]
    pj = pjT.reshape(2, 128, 512).transpose(1, 0, 2).reshape(128, 1024).astype(bf16)

    f64 = lambda k: np.asarray(inputs[k], np.float64)
    fc_wT = {1: f64("fc1_w").T, 2: f64("fc2_w").T, 3: f64("fc3_w").T}

    XTz = np.concatenate([XT, np.zeros((128, NN), np.float32)], axis=0)

    in_maps = []
    for c in range(NCORES):
        # xh: 5 blocks (4 local + halo), stride 264, ones col at 256
        xh = np.zeros((128, 5 * XHS), np.float32)
        for bl in range(5):
            gb = 4 * c + bl
            xh[:, XHS * bl: XHS * bl + 256] = XTz[128 * gb:128 * gb + 128]
            xh[:, XHS * bl + 256] = 1.0
        aux = np.zeros((128, BPC * 3), np.float32)
        for bl in range(BPC):
            gb = 4 * c + bl
            aux[:, 3 * bl + 0] = 1.0
            aux[:, 3 * bl + 1] = g_p[128 * gb:128 * gb + 128]
            aux[:, 3 * bl + 2] = g_n[128 * gb:128 * gb + 128]
        hs = slice(HSH * c, HSH * (c + 1))
        w1 = fc_wT[1][:, hs].reshape(KB1, 128, HSH).transpose(1, 0, 2) \
            .reshape(128, KB1 * HSH).astype(bf16)
        w2full = fc_wT[2][:, hs].reshape(KB, 128, HSH)
        w3full = fc_wT[3][:, hs].reshape(KB, 128, HSH)
        lb = slice(BPC * c, BPC * (c + 1))       # this core's local k-blocks
        w2l = w2full[lb].transpose(1, 0, 2).reshape(128, BPC * HSH).astype(bf16)
        w3l = w3full[lb].transpose(1, 0, 2).reshape(128, BPC * HSH).astype(bf16)
        w2full = w2full.copy(); w2full[lb] = 0.0
        w3full = w3full.copy(); w3full[lb] = 0.0
        w2 = w2full.transpose(1, 0, 2).reshape(128, KB * HSH).astype(bf16)
        w3 = w3full.transpose(1, 0, 2).reshape(128, KB * HSH).astype(bf16)
        b1 = f64("fc1_b")[hs].reshape(4, 128).T.astype(np.float32)
        b2 = f64("fc2_b")[hs].reshape(4, 128).T.astype(np.float32)
        b3 = f64("fc3_b")[hs].reshape(1, HSH).astype(bf16)
        in_maps.append({
            "xt": xt, "xh": xh.astype(bf16), "tp": tp,
            "aux": aux.astype(bf16), "w1": w1, "w2": w2, "w3": w3,
            "w2l": w2l, "w3l": w3l,
            "pj": pj, "b1": b1, "b2": b2, "b3": b3,
        })
    return in_maps, gamma


def run(inputs, trace=False, **kw):
    in_maps, gamma = prep_in_maps(inputs)
    key = ("nc", float(gamma))
    if key not in _CACHE:
        _CACHE[key] = build(gamma)
    nc = _CACHE[key]
    from concourse import bass_utils
    res = bass_utils.run_bass_kernel_spmd(nc, in_maps,
                                          core_ids=list(range(NCORES)),
                                          trace=trace, **kw)
    out = np.concatenate([res.results[c]["out"] for c in range(NCORES)], axis=1)
    return out.astype(np.float32), res


def kernel(**inputs) -> np.ndarray:
    out, _ = run(inputs)
    return out


if __name__ == "__main__":
    data = np.load("inputs.npz")
    inputs = {k: data[k] for k in data.files}
    expected = np.load("expected.npy")
    out = kernel(**inputs)
    scale = np.abs(expected).max()
    err = np.abs(out - expected).max() / scale
    print(f"Relative error: {err:.3e}")



# revision 8
# speedup vs baseline: 1.0045x; 1.0045x over previous
"""Trainium2 Bass kernel for nn_Model1 (lag-weighted long-run covariance + MLP).

Math: the 129-lag weighted covariance collapses algebraically:
    sum_l w_l * (Xc @ Y_l.T) = Xc @ (sum_l w_l Y_l).T
so cov*d = Xc @ P.T + N @ Xc.T with P, N two 65-tap causal FIR filters of Xc.
Centering is pushed through the filters as rank-1 corrections so all GEMMs
run on UNCENTERED X:
    cov.T = U.T/d - m (x) alpha - beta (x) m
with U = X@P0.T + N0@X.T (P0,N0 = filters of raw X), m = row means, and
alpha/beta linear in 5 reduction vectors (r,a,c,p,q) that ride along as
extra rows of the U AllReduce.  (m,alpha,beta) = L @ [r;a;c;p;q] for a
constant 5x3 matrix L shipped as a tiny input.

Distribution (8 cores):
  - cov stage: shard time axis (512 cols/core), one bf16 AllReduce of
    [261,256], triggered as early as possible (it doubles as the inter-core
    rendezvous, absorbing launch skew).
  - MLP: tensor-parallel over hidden (512/core).  The activation AllGathers
    between fc1->fc2 and fc2->fc3 are split in 2 chunks each so the second
    chunk's transfer overlaps the first chunk's matmuls.
  - fc3 emits batch-major so proj shards the output columns; final gather is
    a host-side concat.
All heavy GEMMs use bf16 operands with fp32 PSUM accumulation.  All weights
stream during the AllReduce window so post-AR compute is never DMA-gated.
"""
import math
import numpy as np
import ml_dtypes

NCORES = 8
Q = 64
NN = 256          # n (batch/rows of X)
DD = 4096         # d (time axis)
HID = 4096
Y0 = 512
HSH = HID // NCORES    # 512 hidden shard per core
NBLK = DD // 128       # 32 time blocks
BPC = NBLK // NCORES   # 4 blocks per core
KB = HID // 128        # 32 fc2/fc3 contraction blocks

bf16 = ml_dtypes.bfloat16

_CACHE = {}


# ----------------------------------------------------------------------------
# host-side weight-only precompute
# ----------------------------------------------------------------------------
def _erf(x):
    return np.vectorize(math.erf, otypes=[np.float64])(x)


def _gelu64(x):
    return 0.5 * x * (1.0 + _erf(x / np.sqrt(2.0)))


def _filters(inputs):
    f64 = lambda k: np.asarray(inputs[k], np.float64)
    lags = np.arange(-Q, Q + 1, dtype=np.float64)[:, None]
    h = _gelu64(lags @ f64("wn_w1") + f64("wn_b1"))
    w = (h @ f64("wn_w2") + f64("wn_b2"))[:, 0]
    wp = w[Q:]                                    # l = 0..Q
    wnv = np.concatenate([[0.0], w[:Q][::-1]])    # wnv[l] = w[Q-l], l=1..Q
    v = np.arange(128)[:, None]
    u = np.arange(128)[None, :]
    dvu = v - u
    d2 = dvu + 128
    A0 = np.where((dvu >= 0) & (dvu <= Q), wp[np.clip(dvu, 0, Q)], 0.0)
    A1 = np.where((d2 >= 0) & (d2 <= Q), wp[np.clip(d2, 0, Q)], 0.0)
    B0 = np.where((dvu >= 1) & (dvu <= Q), wnv[np.clip(dvu, 0, Q)], 0.0)
    B1 = np.where((d2 >= 1) & (d2 <= Q), wnv[np.clip(d2, 0, Q)], 0.0)
    t = np.arange(DD)
    lim = np.minimum(Q, DD - 1 - t)
    g_p = np.cumsum(wp)[lim]
    g_n = np.cumsum(wnv)[lim]
    gamma = float(g_p.sum() + g_n.sum())
    return A0, A1, B0, B1, g_p, g_n, gamma


# ----------------------------------------------------------------------------
# bass program
# ----------------------------------------------------------------------------
def build(use_gelu=True):
    import concourse.bacc as bacc
    import concourse.tile as tile
    import concourse.mybir as mybir

    dt32 = mybir.dt.float32
    dt16 = mybir.dt.bfloat16
    GELU = (mybir.ActivationFunctionType.Gelu if use_gelu
            else mybir.ActivationFunctionType.Identity)

    nc = bacc.Bacc("TRN2", target_bir_lowering=False, debug=False,
                   num_devices=NCORES)
    mm = nc.tensor.matmul
    RG = [list(range(NCORES))]

    # ---- I/O ----
    xh_d = nc.dram_tensor("xh", [128, 5 * 256], dt16, kind="ExternalInput").ap()
    tp_d = nc.dram_tensor("tp", [128, 512], dt16, kind="ExternalInput").ap()
    aux_d = nc.dram_tensor("aux", [128, BPC * 3], dt16, kind="ExternalInput").ap()
    lm_d = nc.dram_tensor("lm", [5, 4], dt16, kind="ExternalInput").ap()
    xt_d = nc.dram_tensor("xt", [128, NBLK * 256], dt16, kind="ExternalInput").ap()
    w1_d = nc.dram_tensor("w1", [128, 64 * 512], dt16, kind="ExternalInput").ap()
    w2_d = nc.dram_tensor("w2", [128, KB * 512], dt16, kind="ExternalInput").ap()
    w3_d = nc.dram_tensor("w3", [128, KB * 512], dt16, kind="ExternalInput").ap()
    pj_d = nc.dram_tensor("pj", [128, 2 * 512], dt16, kind="ExternalInput").ap()
    b1_d = nc.dram_tensor("b1", [128, 4], dt32, kind="ExternalInput").ap()
    b2_d = nc.dram_tensor("b2", [128, 4], dt32, kind="ExternalInput").ap()
    b3_d = nc.dram_tensor("b3", [1, 512], dt16, kind="ExternalInput").ap()
    out_d = nc.dram_tensor("out", [Y0, HSH], dt32, kind="ExternalOutput").ap()

    with tile.TileContext(nc) as tc:
        with (
            tc.tile_pool(name="cst", bufs=1) as cst,
            tc.tile_pool(name="pn", bufs=3) as pnp,
            tc.tile_pool(name="wst", bufs=3) as wst,
            tc.tile_pool(name="psA", bufs=1, space="PSUM") as psA,
            tc.tile_pool(name="psB", bufs=2, space="PSUM") as psB,
            tc.tile_pool(name="dram", bufs=1, space="DRAM") as drp,
        ):
            # ---------- early small DMAs (stage-1 dependencies first) ----------
            xh_t = cst.tile([128, 5 * 256], dt16, tag="xh")
            nc.sync.dma_start(xh_t, xh_d)
            tp_t = cst.tile([128, 512], dt16, tag="tp")
            nc.scalar.dma_start(tp_t, tp_d)
            aux_t = cst.tile([128, BPC * 3], dt16, tag="aux")
            nc.scalar.dma_start(aux_t, aux_d)
            lm_t = cst.tile([5, 4], dt16, tag="lm")
            nc.scalar.dma_start(lm_t, lm_d)
            b1_t = cst.tile([128, 4], dt32, tag="b1")
            nc.scalar.dma_start(b1_t, b1_d)
            b2_t = cst.tile([128, 4], dt32, tag="b2")
            nc.scalar.dma_start(b2_t, b2_d)
            b3_t = cst.tile([1, 512], dt16, tag="b3")
            nc.scalar.dma_start(b3_t, b3_d)
            ones_t = cst.tile([128, 1], dt16, tag="ones")
            nc.vector.memset(ones_t, 1.0)
            onesr_t = cst.tile([1, 128], dt16, tag="onesr")
            nc.vector.memset(onesr_t, 1.0)

            # ---------- bulk streaming (fills the AllReduce shadow) ----------
            xt_t = cst.tile([128, NBLK * 256], dt16, tag="xt")
            for sp in range(2):
                w = NBLK * 256 // 2
                nc.sync.dma_start(xt_t[:, w * sp: w * (sp + 1)],
                                  xt_d[:, w * sp: w * (sp + 1)])
            # w1 streamed through a rotating pool: 4 G-half + 4 X-half chunks
            wg_tiles = []
            for ch in range(4):
                wt = wst.tile([128, 4096], dt16, tag="w", name=f"wg{ch}")
                nc.sync.dma_start(wt, w1_d[:, 4096 * ch: 4096 * (ch + 1)])
                wg_tiles.append(wt)
            wx_tiles = []
            for ch in range(4):
                wt = wst.tile([128, 4096], dt16, tag="w", name=f"wx{ch}")
                nc.scalar.dma_start(
                    wt, w1_d[:, 16384 + 4096 * ch: 16384 + 4096 * (ch + 1)])
                wx_tiles.append(wt)
            w2R = cst.tile([128, KB * 512], dt16, tag="w2R")
            for sp in range(4):
                w = KB * 512 // 4
                nc.sync.dma_start(w2R[:, w * sp: w * (sp + 1)],
                                  w2_d[:, w * sp: w * (sp + 1)])
            w3R = cst.tile([128, KB * 512], dt16, tag="w3R")
            for sp in range(4):
                w = KB * 512 // 4
                nc.scalar.dma_start(w3R[:, w * sp: w * (sp + 1)],
                                    w3_d[:, w * sp: w * (sp + 1)])
            pj_t = cst.tile([128, 2 * 512], dt16, tag="pj")
            nc.scalar.dma_start(pj_t, pj_d)

            # bounce buffers (DRAM)
            arA_i = drp.tile([261, NN], dt16, tag="arA_i")
            arA_o = drp.tile([261, NN], dt16, tag="arA_o", addr_space="Shared")
            g1_i = [drp.tile([256, NN], dt16, tag=f"g1{c}_i", name=f"g1{c}_i")
                    for c in range(2)]
            g1_o = [drp.tile([2048, NN], dt16, tag=f"g1{c}_o", name=f"g1{c}_o",
                             addr_space="Shared") for c in range(2)]
            g2_i = [drp.tile([256, NN], dt16, tag=f"g2{c}_i", name=f"g2{c}_i")
                    for c in range(2)]
            g2_o = [drp.tile([2048, NN], dt16, tag=f"g2{c}_o", name=f"g2{c}_o",
                             addr_space="Shared") for c in range(2)]

            # ---------- stage 1: cov partials over local time blocks ----------
            # u_ps[ic]: U.T chunk rows [128*ic, 128*ic+128); rac rows (r,a,c);
            # pq row = [p | q] (column sums of the P/N filter outputs)
            u_ps = [psA.tile([128, 256], dt32, tag=f"acc{ic}", name=f"u{ic}")
                    for ic in range(2)]
            rac_ps = psA.tile([3, 256], dt32, tag="acc2", name="rac_ps")
            pq_ps = psA.tile([1, 512], dt32, tag="acc3", name="pq_ps")
            for bl in range(BPC):
                xb = xh_t[:, 256 * bl: 256 * bl + 256]
                xb1 = xh_t[:, 256 * (bl + 1): 256 * (bl + 1) + 256]
                pn = pnp.tile([128, 512], dt16, tag="pn", name="pn")
                pt_ps = psB.tile([128, 256], dt32, tag="rot", name="pt_ps")
                mm(pt_ps, tp_t[:, 0:128], xb, start=True, stop=False)
                mm(pt_ps, tp_t[:, 128:256], xb1, start=False, stop=True)
                nc.vector.tensor_copy(pn[:, 0:256], pt_ps)
                nt_ps = psB.tile([128, 256], dt32, tag="rot", name="nt_ps")
                mm(nt_ps, tp_t[:, 256:384], xb, start=True, stop=False)
                mm(nt_ps, tp_t[:, 384:512], xb1, start=False, stop=True)
                nc.vector.tensor_copy(pn[:, 256:512], nt_ps)
                first, last = bl == 0, bl == BPC - 1
                for ic in range(2):
                    xbc = xh_t[:, 256 * bl + 128 * ic: 256 * bl + 128 * ic + 128]
                    mm(u_ps[ic], pn[:, 128 * ic:128 * ic + 128], xb,
                       start=first, stop=False)
                    mm(u_ps[ic], xbc, pn[:, 256:512], start=False, stop=last)
                mm(rac_ps, aux_t[:, 3 * bl:3 * bl + 3], xb,
                   start=first, stop=last)
                mm(pq_ps, ones_t, pn[:, 0:512], start=first, stop=last)

            # pack (bf16) + stage + AllReduce (doubles as the rendezvous)
            stgs = []
            for ic in range(2):
                stg = cst.tile([128, 256], dt16, tag=f"stg{ic}", name=f"stg{ic}")
                nc.vector.tensor_scalar_mul(stg, u_ps[ic], 1.0 / DD)
                stgs.append(stg)
            vrac = cst.tile([3, 256], dt16, tag="vrac", name="vrac")
            nc.vector.tensor_copy(vrac, rac_ps)
            vpq = cst.tile([1, 512], dt16, tag="vpq", name="vpq")
            nc.vector.tensor_copy(vpq, pq_ps)
            nc.scalar.dma_start(arA_i[0:128, :], stgs[0])
            nc.scalar.dma_start(arA_i[128:256, :], stgs[1])
            nc.scalar.dma_start(arA_i[256:259, :], vrac)
            nc.scalar.dma_start(
                arA_i[259:261, :].rearrange("(b p) n -> b p n", p=1)
                .transpose([1, 0, 2]),
                vpq.rearrange("p (b n) -> p b n", b=2))
            nc.gpsimd.collective_compute(
                "AllReduce", mybir.AluOpType.add, replica_groups=RG,
                ins=[arA_i.opt()], outs=[arA_o.opt()])

            # ---------- G^T = X @ W1c (cov-independent, in AR shadow) ----------
            g_ps = [psA.tile([128, 512], dt32, tag=f"acc{4 + ib}", name=f"g_{ib}")
                    for ib in range(2)]
            for ch in range(4):
                for kl in range(8):
                    k = 8 * ch + kl
                    for ib in range(2):
                        mm(g_ps[ib],
                           xt_t[:, 256 * k + 128 * ib: 256 * k + 128 * ib + 128],
                           wg_tiles[ch][:, 512 * kl: 512 * kl + 512],
                           start=(k == 0), stop=(k == NBLK - 1))
            gT = cst.tile([128, 2 * 512], dt16, tag="gT")
            for ib in range(2):
                nc.vector.tensor_copy(gT[:, 512 * ib:512 * ib + 512], g_ps[ib])

            # ---------- fc1 X-half (also in AR shadow) ----------
            f1_ps = [psA.tile([128, 256], dt32, tag=f"acc{hh}", name=f"f1_{hh}")
                     for hh in range(4)]
            for ch in range(4):
                for kl in range(8):
                    k = 8 * ch + kl
                    for hh in range(4):
                        mm(f1_ps[hh],
                           wx_tiles[ch][:, 512 * kl + 128 * hh: 512 * kl + 128 * hh + 128],
                           xt_t[:, 256 * k:256 * k + 256],
                           start=(k == 0), stop=False)

            # ---------- post-AR: m/alpha/beta + cov.T ----------
            ured = cst.tile([128, 512], dt16, tag="ured", name="ured")
            nc.sync.dma_start(
                ured.rearrange("p (b n) -> p b n", b=2),
                arA_o[0:256, :].rearrange("(b p) n -> b p n", p=128)
                .transpose([1, 0, 2]))
            rows = cst.tile([5, 256], dt16, tag="rows", name="rows")
            nc.sync.dma_start(rows, arA_o[256:261, :])
            # m/alpha/beta as three 256-col segments of one partition-0 row
            ma_ps = psB.tile([1, 512], dt32, tag="rot", name="ma_ps")
            for s in range(2):
                mm(ma_ps[0:1, 256 * s: 256 * s + 256], lm_t[:, s:s + 1], rows,
                   start=True, stop=True)
            be_ps = psB.tile([1, 256], dt32, tag="rot", name="be_ps")
            mm(be_ps, lm_t[:, 2:3], rows, start=True, stop=True)
            mab = cst.tile([1, 3 * 256], dt16, tag="mab", name="mab")
            nc.vector.tensor_copy(mab[0:1, 0:512], ma_ps)
            nc.vector.tensor_copy(mab[0:1, 512:768], be_ps)
            covt = cst.tile([128, 2 * 256], dt16, tag="covt")
            for ic in range(2):
                corr = psB.tile([128, 256], dt32, tag="rot", name="corr")
                mm(corr, mab[0:1, 128 * ic:128 * ic + 128], mab[0:1, 256:512],
                   start=True, stop=False)
                mm(corr, mab[0:1, 512 + 128 * ic:512 + 128 * ic + 128],
                   mab[0:1, 0:256], start=False, stop=True)
                nc.vector.tensor_sub(covt[:, 256 * ic:256 * ic + 256],
                                     ured[:, 256 * ic:256 * ic + 256], corr)

            # ---------- fc1 cov contribution + gelu + chunked AllGather ----------
            a1loc = cst.tile([128, 4 * 256], dt16, tag="a1loc")
            for half in range(2):
                for hh in (2 * half, 2 * half + 1):
                    for ib in range(2):
                        mm(f1_ps[hh],
                           gT[:, 512 * ib + 128 * hh: 512 * ib + 128 * hh + 128],
                           covt[:, 256 * ib:256 * ib + 256],
                           start=False, stop=(ib == 1))
                    nc.scalar.activation(a1loc[:, 256 * hh:256 * hh + 256],
                                         f1_ps[hh], GELU, bias=b1_t[:, hh:hh + 1])
                nc.scalar.dma_start(
                    g1_i[half].rearrange("(b p) n -> b p n", p=128)
                    .transpose([1, 0, 2]),
                    a1loc[:, 512 * half: 512 * half + 512]
                    .rearrange("p (b n) -> p b n", b=2))
                nc.gpsimd.collective_compute(
                    "AllGather", mybir.AluOpType.bypass, replica_groups=RG,
                    ins=[g1_i[half].opt()], outs=[g1_o[half].opt()])

            # ---------- fc2 (chunked over the two gathers) ----------
            a1f = [cst.tile([128, 16 * 256], dt16, tag=f"a1f{c}", name=f"a1f{c}")
                   for c in range(2)]
            for c in range(2):
                src = (g1_o[c].rearrange("(b p) n -> b p n", p=128)
                       .transpose([1, 0, 2]))
                dst = a1f[c].rearrange("p (b n) -> p b n", b=16)
                eng = nc.sync if c == 0 else nc.scalar
                eng.dma_start(dst[:, 0:8, :], src[:, 0:8, :])
                eng2 = nc.scalar if c == 0 else nc.sync
                eng2.dma_start(dst[:, 8:16, :], src[:, 8:16, :])
            f2_ps = [psA.tile([128, 256], dt32, tag=f"acc{hh}", name=f"f2_{hh}")
                     for hh in range(4)]
            for j in range(16):                      # chunk A, all hh
                for hh in range(4):
                    mm(f2_ps[hh],
                       w2R[:, 512 * j + 128 * hh: 512 * j + 128 * hh + 128],
                       a1f[0][:, 256 * j:256 * j + 256],
                       start=(j == 0), stop=False)
            a2loc = cst.tile([128, 4 * 256], dt16, tag="a2loc")
            for half in range(2):                    # chunk B, by hh-half
                for j in range(16):
                    for hh in (2 * half, 2 * half + 1):
                        mm(f2_ps[hh],
                           w2R[:, 512 * (16 + j) + 128 * hh: 512 * (16 + j) + 128 * hh + 128],
                           a1f[1][:, 256 * j:256 * j + 256],
                           start=False, stop=(j == 15))
                for hh in (2 * half, 2 * half + 1):
                    nc.scalar.activation(a2loc[:, 256 * hh:256 * hh + 256],
                                         f2_ps[hh], GELU, bias=b2_t[:, hh:hh + 1])
                nc.scalar.dma_start(
                    g2_i[half].rearrange("(b p) n -> b p n", p=128)
                    .transpose([1, 0, 2]),
                    a2loc[:, 512 * half: 512 * half + 512]
                    .rearrange("p (b n) -> p b n", b=2))
                nc.gpsimd.collective_compute(
                    "AllGather", mybir.AluOpType.bypass, replica_groups=RG,
                    ins=[g2_i[half].opt()], outs=[g2_o[half].opt()])

            # ---------- fc3 (batch-major out, chunked) ----------
            a2f = [cst.tile([128, 16 * 256], dt16, tag=f"a2f{c}", name=f"a2f{c}")
                   for c in range(2)]
            for c in range(2):
                src = (g2_o[c].rearrange("(b p) n -> b p n", p=128)
                       .transpose([1, 0, 2]))
                dst = a2f[c].rearrange("p (b n) -> p b n", b=16)
                eng = nc.sync if c == 0 else nc.scalar
                eng.dma_start(dst[:, 0:8, :], src[:, 0:8, :])
                eng2 = nc.scalar if c == 0 else nc.sync
                eng2.dma_start(dst[:, 8:16, :], src[:, 8:16, :])
            f3_ps = [psA.tile([128, 512], dt32, tag=f"acc{4 + ii}", name=f"f3_{ii}")
                     for ii in range(2)]
            o3_t = cst.tile([128, 2 * 512], dt16, tag="o3")
            for c in range(2):
                for j in range(16):
                    for ii in range(2):
                        mm(f3_ps[ii],
                           a2f[c][:, 256 * j + 128 * ii: 256 * j + 128 * ii + 128],
                           w3R[:, 512 * (16 * c + j): 512 * (16 * c + j) + 512],
                           start=(c == 0 and j == 0), stop=False)
            for ii in range(2):
                mm(f3_ps[ii], onesr_t, b3_t, start=False, stop=True)
                nc.vector.tensor_copy(o3_t[:, 512 * ii:512 * ii + 512], f3_ps[ii])

            # ---------- proj ----------
            for pp in range(4):
                po = psB.tile([128, 512], dt32, tag="rot", name="po")
                for ii in range(2):
                    mm(po, pj_t[:, 512 * ii + 128 * pp: 512 * ii + 128 * pp + 128],
                       o3_t[:, 512 * ii:512 * ii + 512],
                       start=(ii == 0), stop=(ii == 1))
                osb = cst.tile([128, 512], dt32, tag=f"osb{pp}", name=f"osb{pp}")
                nc.vector.tensor_copy(osb, po)
                nc.sync.dma_start(out_d[128 * pp:128 * pp + 128, :], osb)

    nc.compile()
    return nc


# ----------------------------------------------------------------------------
# host-side sharding / packing
# ----------------------------------------------------------------------------
def prep_in_maps(inputs):
    X = np.asarray(inputs["X"], np.float32)
    A0, A1, B0, B1, g_p, g_n, gamma = _filters(inputs)

    XT = np.ascontiguousarray(X.T)                      # [D, N]
    xt = XT.reshape(NBLK, 128, NN).transpose(1, 0, 2).reshape(128, NBLK * 256)
    xt = xt.astype(bf16)
    tp = np.concatenate([A0, A1, B0, B1], axis=1).astype(bf16)
    pjT = np.asarray(inputs["proj"], np.float64).T      # [256, 512]
    pj = pjT.reshape(2, 128, 512).transpose(1, 0, 2).reshape(128, 1024).astype(bf16)

    lm = np.zeros((5, 4), np.float64)
    lm[0, 0] = 1.0 / DD                 # m  <- r
    lm[0, 1] = -gamma / DD**2           # al <- r
    lm[1, 1] = 1.0 / DD                 # al <- a
    lm[4, 1] = 1.0 / DD                 # al <- q
    lm[2, 2] = 1.0 / DD                 # be <- c
    lm[3, 2] = 1.0 / DD                 # be <- p
    lm = lm.astype(bf16)

    f64 = lambda k: np.asarray(inputs[k], np.float64)
    fc_wT = {1: f64("fc1_w").T, 2: f64("fc2_w").T, 3: f64("fc3_w").T}

    XTz = np.concatenate([XT, np.zeros((128, NN), np.float32)], axis=0)

    # fc2/fc3 contraction-block order matching the chunked gathers:
    # chunk A = [4r+hh for r in 0..7 for hh in (0,1)], chunk B = hh in (2,3)
    blkorder = ([4 * r + hh for r in range(NCORES) for hh in (0, 1)]
                + [4 * r + hh for r in range(NCORES) for hh in (2, 3)])

    in_maps = []
    for c in range(NCORES):
        xh = np.zeros((128, 5 * 256), np.float32)
        for bl in range(5):
            gb = 4 * c + bl
            xh[:, 256 * bl: 256 * bl + 256] = XTz[128 * gb:128 * gb + 128]
        aux = np.zeros((128, BPC * 3), np.float32)
        for bl in range(BPC):
            gb = 4 * c + bl
            aux[:, 3 * bl + 0] = 1.0
            aux[:, 3 * bl + 1] = g_p[128 * gb:128 * gb + 128]
            aux[:, 3 * bl + 2] = g_n[128 * gb:128 * gb + 128]
        hs = slice(HSH * c, HSH * (c + 1))
        # w1: G-half (cov rows 4096..8191) first, then X-half
        w1rows = np.concatenate([fc_wT[1][DD:, hs], fc_wT[1][:DD, hs]], axis=0)
        w1 = w1rows.reshape(64, 128, HSH).transpose(1, 0, 2) \
            .reshape(128, 64 * HSH).astype(bf16)
        w2 = fc_wT[2][:, hs].reshape(KB, 128, HSH)[blkorder] \
            .transpose(1, 0, 2).reshape(128, KB * HSH).astype(bf16)
        w3 = fc_wT[3][:, hs].reshape(KB, 128, HSH)[blkorder] \
            .transpose(1, 0, 2).reshape(128, KB * HSH).astype(bf16)
        b1 = f64("fc1_b")[hs].reshape(4, 128).T.astype(np.float32)
        b2 = f64("fc2_b")[hs].reshape(4, 128).T.astype(np.float32)
        b3 = f64("fc3_b")[hs].reshape(1, HSH).astype(bf16)
        in_maps.append({
            "xt": xt, "xh": xh.astype(bf16), "tp": tp, "lm": lm,
            "aux": aux.astype(bf16), "w1": w1, "w2": w2, "w3": w3,
            "pj": pj, "b1": b1, "b2": b2, "b3": b3,
        })
    return in_maps


def run(inputs, trace=False, **kw):
    in_maps = prep_in_maps(inputs)
    if "nc" not in _CACHE:
        _CACHE["nc"] = build()
    nc = _CACHE["nc"]
    from concourse import bass_utils
    res = bass_utils.run_bass_kernel_spmd(nc, in_maps,
                                          core_ids=list(range(NCORES)),
                                          trace=trace, **kw)
    out = np.concatenate([res.results[c]["out"] for c in range(NCORES)], axis=1)
    return out.astype(np.float32), res


def kernel(**inputs) -> np.ndarray:
    out, _ = run(inputs)
    return out


if __name__ == "__main__":
    data = np.load("inputs.npz")
    inputs = {k: data[k] for k in data.files}
    expected = np.load("expected.npy")
    out = kernel(**inputs)
    scale = np.abs(expected).max()
    err = np.abs(out - expected).max() / scale
    print(f"Relative error: {err:.3e}")


# revision 10
# speedup vs baseline: 1.0501x; 1.0454x over previous
"""Trainium2 Bass kernel for nn_Model1 (lag-weighted long-run covariance + MLP).

Math: the 129-lag weighted covariance collapses algebraically:
    sum_l w_l * (Xc @ Y_l.T) = Xc @ (sum_l w_l Y_l).T
so cov*d = Xc @ P.T + N @ Xc.T with P, N two 65-tap causal FIR filters of Xc.
Centering is pushed through the filters as rank-1 corrections so all GEMMs
run on UNCENTERED X:
    cov.T = U.T/d - m (x) alpha - beta (x) m
with U = X@P0.T + N0@X.T (P0,N0 = filters of raw X), m = row means, and
alpha/beta linear in 5 reduction vectors (r,a,c,p,q) that ride along as
extra rows of the U AllReduce.  (m,alpha,beta) = L @ [r;a;c;p;q] for a
constant 5x3 matrix L shipped as a tiny input.

Distribution (8 cores):
  - cov stage: shard time axis (512 cols/core), one bf16 AllReduce of
    [261,256], triggered as early as possible (it doubles as the inter-core
    rendezvous, absorbing launch skew).
  - MLP: tensor-parallel over hidden (512/core).  The activation AllGathers
    between fc1->fc2 and fc2->fc3 are split in 2 chunks each so the second
    chunk's transfer overlaps the first chunk's matmuls.
  - fc3 emits batch-major so proj shards the output columns; final gather is
    a host-side concat.
All heavy GEMMs use bf16 operands with fp32 PSUM accumulation.  All weights
stream during the AllReduce window so post-AR compute is never DMA-gated.
"""
import math
import numpy as np
import ml_dtypes

NCORES = 8
Q = 64
NN = 256          # n (batch/rows of X)
DD = 4096         # d (time axis)
HID = 4096
Y0 = 512
HSH = HID // NCORES    # 512 hidden shard per core
NBLK = DD // 128       # 32 time blocks
BPC = NBLK // NCORES   # 4 blocks per core
KB = HID // 128        # 32 fc2/fc3 contraction blocks

bf16 = ml_dtypes.bfloat16

_CACHE = {}


# ----------------------------------------------------------------------------
# host-side weight-only precompute
# ----------------------------------------------------------------------------
def _erf(x):
    return np.vectorize(math.erf, otypes=[np.float64])(x)


def _gelu64(x):
    return 0.5 * x * (1.0 + _erf(x / np.sqrt(2.0)))


def _filters(inputs):
    f64 = lambda k: np.asarray(inputs[k], np.float64)
    lags = np.arange(-Q, Q + 1, dtype=np.float64)[:, None]
    h = _gelu64(lags @ f64("wn_w1") + f64("wn_b1"))
    w = (h @ f64("wn_w2") + f64("wn_b2"))[:, 0]
    wp = w[Q:]                                    # l = 0..Q
    wnv = np.concatenate([[0.0], w[:Q][::-1]])    # wnv[l] = w[Q-l], l=1..Q
    v = np.arange(128)[:, None]
    u = np.arange(128)[None, :]
    dvu = v - u
    d2 = dvu + 128
    A0 = np.where((dvu >= 0) & (dvu <= Q), wp[np.clip(dvu, 0, Q)], 0.0)
    A1 = np.where((d2 >= 0) & (d2 <= Q), wp[np.clip(d2, 0, Q)], 0.0)
    B0 = np.where((dvu >= 1) & (dvu <= Q), wnv[np.clip(dvu, 0, Q)], 0.0)
    B1 = np.where((d2 >= 1) & (d2 <= Q), wnv[np.clip(d2, 0, Q)], 0.0)
    t = np.arange(DD)
    lim = np.minimum(Q, DD - 1 - t)
    g_p = np.cumsum(wp)[lim]
    g_n = np.cumsum(wnv)[lim]
    gamma = float(g_p.sum() + g_n.sum())
    return A0, A1, B0, B1, g_p, g_n, gamma


# ----------------------------------------------------------------------------
# bass program
# ----------------------------------------------------------------------------
def build(use_gelu=True):
    import concourse.bacc as bacc
    import concourse.tile as tile
    import concourse.mybir as mybir

    dt32 = mybir.dt.float32
    dt16 = mybir.dt.bfloat16
    GELU = (mybir.ActivationFunctionType.Gelu if use_gelu
            else mybir.ActivationFunctionType.Identity)

    nc = bacc.Bacc("TRN2", target_bir_lowering=False, debug=False,
                   num_devices=NCORES)
    mm = nc.tensor.matmul
    RG = [list(range(NCORES))]

    # ---- I/O ----
    xh_d = nc.dram_tensor("xh", [128, 5 * 256], dt16, kind="ExternalInput").ap()
    tp_d = nc.dram_tensor("tp", [128, 512], dt16, kind="ExternalInput").ap()
    aux_d = nc.dram_tensor("aux", [128, BPC * 3], dt16, kind="ExternalInput").ap()
    lm_d = nc.dram_tensor("lm", [5, 4], dt16, kind="ExternalInput").ap()
    xt_d = nc.dram_tensor("xt", [128, NBLK * 256], dt16, kind="ExternalInput").ap()
    w1_d = nc.dram_tensor("w1", [128, 64 * 512], dt16, kind="ExternalInput").ap()
    w2_d = nc.dram_tensor("w2", [128, KB * 512], dt16, kind="ExternalInput").ap()
    w3_d = nc.dram_tensor("w3", [128, KB * 512], dt16, kind="ExternalInput").ap()
    pj_d = nc.dram_tensor("pj", [128, 2 * 512], dt16, kind="ExternalInput").ap()
    b1_d = nc.dram_tensor("b1", [128, 4], dt32, kind="ExternalInput").ap()
    b2_d = nc.dram_tensor("b2", [128, 4], dt32, kind="ExternalInput").ap()
    b3_d = nc.dram_tensor("b3", [1, 512], dt16, kind="ExternalInput").ap()
    out_d = nc.dram_tensor("out", [Y0, HSH], dt32, kind="ExternalOutput").ap()

    with tile.TileContext(nc) as tc:
        with (
            tc.tile_pool(name="cst", bufs=1) as cst,
            tc.tile_pool(name="pn", bufs=3) as pnp,
            tc.tile_pool(name="wst", bufs=3) as wst,
            tc.tile_pool(name="psA", bufs=1, space="PSUM") as psA,
            tc.tile_pool(name="psB", bufs=2, space="PSUM") as psB,
            tc.tile_pool(name="dram", bufs=1, space="DRAM") as drp,
        ):
            # ---------- early small DMAs (stage-1 dependencies first) ----------
            xh_t = cst.tile([128, 5 * 256], dt16, tag="xh")
            nc.sync.dma_start(xh_t, xh_d)
            tp_t = cst.tile([128, 512], dt16, tag="tp")
            nc.scalar.dma_start(tp_t, tp_d)
            aux_t = cst.tile([128, BPC * 3], dt16, tag="aux")
            nc.scalar.dma_start(aux_t, aux_d)
            lm_t = cst.tile([5, 4], dt16, tag="lm")
            nc.scalar.dma_start(lm_t, lm_d)
            b1_t = cst.tile([128, 4], dt32, tag="b1")
            nc.scalar.dma_start(b1_t, b1_d)
            b2_t = cst.tile([128, 4], dt32, tag="b2")
            nc.scalar.dma_start(b2_t, b2_d)
            b3_t = cst.tile([1, 512], dt16, tag="b3")
            nc.scalar.dma_start(b3_t, b3_d)
            ones_t = cst.tile([128, 1], dt16, tag="ones")
            nc.vector.memset(ones_t, 1.0)
            onesr_t = cst.tile([1, 128], dt16, tag="onesr")
            nc.vector.memset(onesr_t, 1.0)

            # ---------- bulk streaming (fills the AllReduce shadow) ----------
            xt_t = cst.tile([128, NBLK * 256], dt16, tag="xt")
            for sp in range(2):
                w = NBLK * 256 // 2
                nc.sync.dma_start(xt_t[:, w * sp: w * (sp + 1)],
                                  xt_d[:, w * sp: w * (sp + 1)])
            # w1 streamed through a rotating pool: 4 G-half + 4 X-half chunks
            wg_tiles = []
            for ch in range(4):
                wt = wst.tile([128, 4096], dt16, tag="w", name=f"wg{ch}")
                nc.sync.dma_start(wt, w1_d[:, 4096 * ch: 4096 * (ch + 1)])
                wg_tiles.append(wt)
            wx_tiles = []
            for ch in range(4):
                wt = wst.tile([128, 4096], dt16, tag="w", name=f"wx{ch}")
                nc.scalar.dma_start(
                    wt, w1_d[:, 16384 + 4096 * ch: 16384 + 4096 * (ch + 1)])
                wx_tiles.append(wt)
            w2R = cst.tile([128, KB * 512], dt16, tag="w2R")
            for sp in range(4):
                w = KB * 512 // 4
                nc.sync.dma_start(w2R[:, w * sp: w * (sp + 1)],
                                  w2_d[:, w * sp: w * (sp + 1)])
            w3R = cst.tile([128, KB * 512], dt16, tag="w3R")
            for sp in range(4):
                w = KB * 512 // 4
                nc.scalar.dma_start(w3R[:, w * sp: w * (sp + 1)],
                                    w3_d[:, w * sp: w * (sp + 1)])
            pj_t = cst.tile([128, 2 * 512], dt16, tag="pj")
            nc.scalar.dma_start(pj_t, pj_d)

            # bounce buffers (DRAM)
            arA_i = drp.tile([261, NN], dt16, tag="arA_i")
            arA_o = drp.tile([261, NN], dt16, tag="arA_o", addr_space="Shared")
            g1_i = drp.tile([128, 1024], dt16, tag="g1_i", name="g1_i")
            g1_o = drp.tile([1024, 1024], dt16, tag="g1_o", name="g1_o",
                            addr_space="Shared")
            g2_i = drp.tile([128, 1024], dt16, tag="g2_i", name="g2_i")
            g2_o = drp.tile([1024, 1024], dt16, tag="g2_o", name="g2_o",
                            addr_space="Shared")

            # ---------- stage 1: cov partials over local time blocks ----------
            # u_ps[ic]: U.T chunk rows [128*ic, 128*ic+128); rac rows (r,a,c);
            # pq row = [p | q] (column sums of the P/N filter outputs)
            u_ps = [psA.tile([128, 256], dt32, tag=f"acc{ic}", name=f"u{ic}")
                    for ic in range(2)]
            rac_ps = psA.tile([3, 256], dt32, tag="acc2", name="rac_ps")
            pq_ps = psA.tile([1, 512], dt32, tag="acc3", name="pq_ps")
            for bl in range(BPC):
                xb = xh_t[:, 256 * bl: 256 * bl + 256]
                xb1 = xh_t[:, 256 * (bl + 1): 256 * (bl + 1) + 256]
                pn = pnp.tile([128, 512], dt16, tag="pn", name="pn")
                pt_ps = psB.tile([128, 256], dt32, tag="rot", name="pt_ps")
                mm(pt_ps, tp_t[:, 0:128], xb, start=True, stop=False)
                mm(pt_ps, tp_t[:, 128:256], xb1, start=False, stop=True)
                nc.vector.tensor_copy(pn[:, 0:256], pt_ps)
                nt_ps = psB.tile([128, 256], dt32, tag="rot", name="nt_ps")
                mm(nt_ps, tp_t[:, 256:384], xb, start=True, stop=False)
                mm(nt_ps, tp_t[:, 384:512], xb1, start=False, stop=True)
                nc.vector.tensor_copy(pn[:, 256:512], nt_ps)
                first, last = bl == 0, bl == BPC - 1
                for ic in range(2):
                    xbc = xh_t[:, 256 * bl + 128 * ic: 256 * bl + 128 * ic + 128]
                    mm(u_ps[ic], pn[:, 128 * ic:128 * ic + 128], xb,
                       start=first, stop=False)
                    mm(u_ps[ic], xbc, pn[:, 256:512], start=False, stop=last)
                mm(rac_ps, aux_t[:, 3 * bl:3 * bl + 3], xb,
                   start=first, stop=last)
                mm(pq_ps, ones_t, pn[:, 0:512], start=first, stop=last)

            # pack (bf16) + stage + AllReduce (doubles as the rendezvous)
            stgs = []
            for ic in range(2):
                stg = cst.tile([128, 256], dt16, tag=f"stg{ic}", name=f"stg{ic}")
                nc.vector.tensor_scalar_mul(stg, u_ps[ic], 1.0 / DD)
                stgs.append(stg)
            vrac = cst.tile([3, 256], dt16, tag="vrac", name="vrac")
            nc.vector.tensor_copy(vrac, rac_ps)
            vpq = cst.tile([1, 512], dt16, tag="vpq", name="vpq")
            nc.vector.tensor_copy(vpq, pq_ps)
            nc.scalar.dma_start(arA_i[0:128, :], stgs[0])
            nc.scalar.dma_start(arA_i[128:256, :], stgs[1])
            nc.scalar.dma_start(arA_i[256:259, :], vrac)
            nc.scalar.dma_start(
                arA_i[259:261, :].rearrange("(b p) n -> b p n", p=1)
                .transpose([1, 0, 2]),
                vpq.rearrange("p (b n) -> p b n", b=2))
            nc.gpsimd.collective_compute(
                "AllReduce", mybir.AluOpType.add, replica_groups=RG,
                ins=[arA_i.opt()], outs=[arA_o.opt()])

            # ---------- G^T = X @ W1c (cov-independent, in AR shadow) ----------
            g_ps = [psA.tile([128, 512], dt32, tag=f"acc{4 + ib}", name=f"g_{ib}")
                    for ib in range(2)]
            for ch in range(4):
                for kl in range(8):
                    k = 8 * ch + kl
                    for ib in range(2):
                        mm(g_ps[ib],
                           xt_t[:, 256 * k + 128 * ib: 256 * k + 128 * ib + 128],
                           wg_tiles[ch][:, 512 * kl: 512 * kl + 512],
                           start=(k == 0), stop=(k == NBLK - 1))
            gT = cst.tile([128, 2 * 512], dt16, tag="gT")
            for ib in range(2):
                nc.vector.tensor_copy(gT[:, 512 * ib:512 * ib + 512], g_ps[ib])

            # ---------- fc1 X-half (also in AR shadow) ----------
            f1_ps = [psA.tile([128, 256], dt32, tag=f"acc{hh}", name=f"f1_{hh}")
                     for hh in range(4)]
            for ch in range(4):
                for kl in range(8):
                    k = 8 * ch + kl
                    for hh in range(4):
                        mm(f1_ps[hh],
                           wx_tiles[ch][:, 512 * kl + 128 * hh: 512 * kl + 128 * hh + 128],
                           xt_t[:, 256 * k:256 * k + 256],
                           start=(k == 0), stop=False)

            # ---------- post-AR: m/alpha/beta + cov.T ----------
            ured = cst.tile([128, 512], dt16, tag="ured", name="ured")
            nc.sync.dma_start(
                ured.rearrange("p (b n) -> p b n", b=2),
                arA_o[0:256, :].rearrange("(b p) n -> b p n", p=128)
                .transpose([1, 0, 2]))
            rows = cst.tile([5, 256], dt16, tag="rows", name="rows")
            nc.sync.dma_start(rows, arA_o[256:261, :])
            # m/alpha/beta as three 256-col segments of one partition-0 row
            ma_ps = psB.tile([1, 512], dt32, tag="rot", name="ma_ps")
            for s in range(2):
                mm(ma_ps[0:1, 256 * s: 256 * s + 256], lm_t[:, s:s + 1], rows,
                   start=True, stop=True)
            be_ps = psB.tile([1, 256], dt32, tag="rot", name="be_ps")
            mm(be_ps, lm_t[:, 2:3], rows, start=True, stop=True)
            mab = cst.tile([1, 3 * 256], dt16, tag="mab", name="mab")
            nc.vector.tensor_copy(mab[0:1, 0:512], ma_ps)
            nc.vector.tensor_copy(mab[0:1, 512:768], be_ps)
            covt = cst.tile([128, 2 * 256], dt16, tag="covt")
            for ic in range(2):
                corr = psB.tile([128, 256], dt32, tag="rot", name="corr")
                mm(corr, mab[0:1, 128 * ic:128 * ic + 128], mab[0:1, 256:512],
                   start=True, stop=False)
                mm(corr, mab[0:1, 512 + 128 * ic:512 + 128 * ic + 128],
                   mab[0:1, 0:256], start=False, stop=True)
                nc.vector.tensor_sub(covt[:, 256 * ic:256 * ic + 256],
                                     ured[:, 256 * ic:256 * ic + 256], corr)

            # ---------- fc1 cov contribution + gelu + chunked AllGather ----------
            a1loc = cst.tile([128, 4 * 256], dt16, tag="a1loc")
            for hh in range(4):
                for ib in range(2):
                    mm(f1_ps[hh],
                       gT[:, 512 * ib + 128 * hh: 512 * ib + 128 * hh + 128],
                       covt[:, 256 * ib:256 * ib + 256],
                       start=False, stop=(ib == 1))
                nc.scalar.activation(a1loc[:, 256 * hh:256 * hh + 256],
                                     f1_ps[hh], GELU, bias=b1_t[:, hh:hh + 1])
            nc.scalar.dma_start(g1_i, a1loc)
            nc.gpsimd.collective_compute(
                "AllGather", mybir.AluOpType.bypass, replica_groups=RG,
                ins=[g1_i.opt()], outs=[g1_o.opt()])

            # ---------- fc2 (chunked over the two gathers) ----------
            a1f = [cst.tile([128, 16 * 256], dt16, tag=f"a1f{c}", name=f"a1f{c}")
                   for c in range(2)]
            src1 = (g1_o.rearrange("(r p) n -> r p n", p=128)
                    .transpose([1, 0, 2]))           # [128, 8, 1024]
            for c in range(2):
                dst = a1f[c].rearrange("p (r n) -> p r n", r=4)
                eng = nc.sync if c == 0 else nc.scalar
                eng.dma_start(dst, src1[:, 4 * c:4 * c + 4, :])
            f2_ps = [psA.tile([128, 256], dt32, tag=f"acc{hh}", name=f"f2_{hh}")
                     for hh in range(4)]
            for j in range(16):                      # first half, all hh
                for hh in range(4):
                    mm(f2_ps[hh],
                       w2R[:, 512 * j + 128 * hh: 512 * j + 128 * hh + 128],
                       a1f[0][:, 256 * j:256 * j + 256],
                       start=(j == 0), stop=False)
            a2loc = cst.tile([128, 4 * 256], dt16, tag="a2loc")
            for hh in range(4):                      # second half hh-major
                for j in range(16):
                    mm(f2_ps[hh],
                       w2R[:, 512 * (16 + j) + 128 * hh: 512 * (16 + j) + 128 * hh + 128],
                       a1f[1][:, 256 * j:256 * j + 256],
                       start=False, stop=(j == 15))
                nc.scalar.activation(a2loc[:, 256 * hh:256 * hh + 256],
                                     f2_ps[hh], GELU, bias=b2_t[:, hh:hh + 1])
            nc.scalar.dma_start(g2_i, a2loc)
            nc.gpsimd.collective_compute(
                "AllGather", mybir.AluOpType.bypass, replica_groups=RG,
                ins=[g2_i.opt()], outs=[g2_o.opt()])

            # ---------- fc3 (batch-major out, chunked) ----------
            a2f = [cst.tile([128, 16 * 256], dt16, tag=f"a2f{c}", name=f"a2f{c}")
                   for c in range(2)]
            src2 = (g2_o.rearrange("(r p) n -> r p n", p=128)
                    .transpose([1, 0, 2]))
            for c in range(2):
                dst = a2f[c].rearrange("p (r n) -> p r n", r=4)
                eng = nc.sync if c == 0 else nc.scalar
                eng.dma_start(dst, src2[:, 4 * c:4 * c + 4, :])
            f3_ps = [psA.tile([128, 512], dt32, tag=f"acc{4 + ii}", name=f"f3_{ii}")
                     for ii in range(2)]
            o3_t = cst.tile([128, 2 * 512], dt16, tag="o3")
            for c in range(2):
                for j in range(16):
                    for ii in range(2):
                        mm(f3_ps[ii],
                           a2f[c][:, 256 * j + 128 * ii: 256 * j + 128 * ii + 128],
                           w3R[:, 512 * (16 * c + j): 512 * (16 * c + j) + 512],
                           start=(c == 0 and j == 0), stop=False)
            for ii in range(2):
                mm(f3_ps[ii], onesr_t, b3_t, start=False, stop=True)

            # ---------- proj (interleaved with o3 evacuation) ----------
            po = [psA.tile([128, 512], dt32, tag=f"acc{pp}", name=f"po{pp}")
                  for pp in range(4)]
            for ii in range(2):
                nc.vector.tensor_copy(o3_t[:, 512 * ii:512 * ii + 512], f3_ps[ii])
                for pp in range(4):
                    mm(po[pp],
                       pj_t[:, 512 * ii + 128 * pp: 512 * ii + 128 * pp + 128],
                       o3_t[:, 512 * ii:512 * ii + 512],
                       start=(ii == 0), stop=(ii == 1))
            for pp in range(4):
                osb = cst.tile([128, 512], dt32, tag=f"osb{pp}", name=f"osb{pp}")
                nc.vector.tensor_copy(osb, po[pp])
                nc.sync.dma_start(out_d[128 * pp:128 * pp + 128, :], osb)

    nc.compile()
    return nc


# ----------------------------------------------------------------------------
# host-side sharding / packing
# ----------------------------------------------------------------------------
def prep_in_maps(inputs):
    X = np.asarray(inputs["X"], np.float32)
    A0, A1, B0, B1, g_p, g_n, gamma = _filters(inputs)

    XT = np.ascontiguousarray(X.T)                      # [D, N]
    xt = XT.reshape(NBLK, 128, NN).transpose(1, 0, 2).reshape(128, NBLK * 256)
    xt = xt.astype(bf16)
    tp = np.concatenate([A0, A1, B0, B1], axis=1).astype(bf16)
    pjT = np.asarray(inputs["proj"], np.float64).T      # [256, 512]
    pj = pjT.reshape(2, 128, 512).transpose(1, 0, 2).reshape(128, 1024).astype(bf16)

    lm = np.zeros((5, 4), np.float64)
    lm[0, 0] = 1.0 / DD                 # m  <- r
    lm[0, 1] = -gamma / DD**2           # al <- r
    lm[1, 1] = 1.0 / DD                 # al <- a
    lm[4, 1] = 1.0 / DD                 # al <- q
    lm[2, 2] = 1.0 / DD                 # be <- c
    lm[3, 2] = 1.0 / DD                 # be <- p
    lm = lm.astype(bf16)

    f64 = lambda k: np.asarray(inputs[k], np.float64)
    fc_wT = {1: f64("fc1_w").T, 2: f64("fc2_w").T, 3: f64("fc3_w").T}

    XTz = np.concatenate([XT, np.zeros((128, NN), np.float32)], axis=0)

    in_maps = []
    for c in range(NCORES):
        xh = np.zeros((128, 5 * 256), np.float32)
        for bl in range(5):
            gb = 4 * c + bl
            xh[:, 256 * bl: 256 * bl + 256] = XTz[128 * gb:128 * gb + 128]
        aux = np.zeros((128, BPC * 3), np.float32)
        for bl in range(BPC):
            gb = 4 * c + bl
            aux[:, 3 * bl + 0] = 1.0
            aux[:, 3 * bl + 1] = g_p[128 * gb:128 * gb + 128]
            aux[:, 3 * bl + 2] = g_n[128 * gb:128 * gb + 128]
        hs = slice(HSH * c, HSH * (c + 1))
        # w1: G-half (cov rows 4096..8191) first, then X-half
        w1rows = np.concatenate([fc_wT[1][DD:, hs], fc_wT[1][:DD, hs]], axis=0)
        w1 = w1rows.reshape(64, 128, HSH).transpose(1, 0, 2) \
            .reshape(128, 64 * HSH).astype(bf16)
        w2 = fc_wT[2][:, hs].reshape(KB, 128, HSH) \
            .transpose(1, 0, 2).reshape(128, KB * HSH).astype(bf16)
        w3 = fc_wT[3][:, hs].reshape(KB, 128, HSH) \
            .transpose(1, 0, 2).reshape(128, KB * HSH).astype(bf16)
        b1 = f64("fc1_b")[hs].reshape(4, 128).T.astype(np.float32)
        b2 = f64("fc2_b")[hs].reshape(4, 128).T.astype(np.float32)
        b3 = f64("fc3_b")[hs].reshape(1, HSH).astype(bf16)
        in_maps.append({
            "xt": xt, "xh": xh.astype(bf16), "tp": tp, "lm": lm,
            "aux": aux.astype(bf16), "w1": w1, "w2": w2, "w3": w3,
            "pj": pj, "b1": b1, "b2": b2, "b3": b3,
        })
    return in_maps


def run(inputs, trace=False, **kw):
    in_maps = prep_in_maps(inputs)
    if "nc" not in _CACHE:
        _CACHE["nc"] = build()
    nc = _CACHE["nc"]
    from concourse import bass_utils
    res = bass_utils.run_bass_kernel_spmd(nc, in_maps,
                                          core_ids=list(range(NCORES)),
                                          trace=trace, **kw)
    out = np.concatenate([res.results[c]["out"] for c in range(NCORES)], axis=1)
    return out.astype(np.float32), res


def kernel(**inputs) -> np.ndarray:
    out, _ = run(inputs)
    return out


if __name__ == "__main__":
    data = np.load("inputs.npz")
    inputs = {k: data[k] for k in data.files}
    expected = np.load("expected.npy")
    out = kernel(**inputs)
    scale = np.abs(expected).max()
    err = np.abs(out - expected).max() / scale
    print(f"Relative error: {err:.3e}")


# revision 11
# speedup vs baseline: 1.1138x; 1.0607x over previous
"""Trainium2 Bass kernel for nn_Model1 (lag-weighted long-run covariance + MLP).

Math: the 129-lag weighted covariance collapses algebraically:
    sum_l w_l * (Xc @ Y_l.T) = Xc @ (sum_l w_l Y_l).T
so cov*d = Xc @ P.T + N @ Xc.T with P, N two 65-tap causal FIR filters of Xc.
Centering is pushed through the filters as rank-1 corrections so all GEMMs
run on UNCENTERED X:
    cov.T = U.T/d - m (x) alpha - beta (x) m
with U = X@P0.T + N0@X.T (P0,N0 = filters of raw X), m = row means, and
alpha/beta linear in 5 reduction vectors (r,a,c,p,q) that ride along as
extra rows of the U AllReduce.  (m,alpha,beta) = L @ [r;a;c;p;q] for a
constant 5x3 matrix L shipped as a tiny input.

Distribution (8 cores):
  - cov stage: shard time axis (512 cols/core), one bf16 AllReduce of
    [261,256], triggered as early as possible (it doubles as the inter-core
    rendezvous, absorbing launch skew).
  - MLP: tensor-parallel over hidden (512/core).  The activation AllGathers
    between fc1->fc2 and fc2->fc3 are split in 2 chunks each so the second
    chunk's transfer overlaps the first chunk's matmuls.
  - fc3 emits batch-major so proj shards the output columns; final gather is
    a host-side concat.
All heavy GEMMs use bf16 operands with fp32 PSUM accumulation.  All weights
stream during the AllReduce window so post-AR compute is never DMA-gated.
"""
import math
import numpy as np
import ml_dtypes

NCORES = 8
Q = 64
NN = 256          # n (batch/rows of X)
DD = 4096         # d (time axis)
HID = 4096
Y0 = 512
HSH = HID // NCORES    # 512 hidden shard per core
NBLK = DD // 128       # 32 time blocks
BPC = NBLK // NCORES   # 4 blocks per core
KB = HID // 128        # 32 fc2/fc3 contraction blocks

bf16 = ml_dtypes.bfloat16

_CACHE = {}


# ----------------------------------------------------------------------------
# host-side weight-only precompute
# ----------------------------------------------------------------------------
def _erf(x):
    return np.vectorize(math.erf, otypes=[np.float64])(x)


def _gelu64(x):
    return 0.5 * x * (1.0 + _erf(x / np.sqrt(2.0)))


def _filters(inputs):
    f64 = lambda k: np.asarray(inputs[k], np.float64)
    lags = np.arange(-Q, Q + 1, dtype=np.float64)[:, None]
    h = _gelu64(lags @ f64("wn_w1") + f64("wn_b1"))
    w = (h @ f64("wn_w2") + f64("wn_b2"))[:, 0]
    wp = w[Q:]                                    # l = 0..Q
    wnv = np.concatenate([[0.0], w[:Q][::-1]])    # wnv[l] = w[Q-l], l=1..Q
    v = np.arange(128)[:, None]
    u = np.arange(128)[None, :]
    dvu = v - u
    d2 = dvu + 128
    A0 = np.where((dvu >= 0) & (dvu <= Q), wp[np.clip(dvu, 0, Q)], 0.0)
    A1 = np.where((d2 >= 0) & (d2 <= Q), wp[np.clip(d2, 0, Q)], 0.0)
    B0 = np.where((dvu >= 1) & (dvu <= Q), wnv[np.clip(dvu, 0, Q)], 0.0)
    B1 = np.where((d2 >= 1) & (d2 <= Q), wnv[np.clip(d2, 0, Q)], 0.0)
    t = np.arange(DD)
    lim = np.minimum(Q, DD - 1 - t)
    g_p = np.cumsum(wp)[lim]
    g_n = np.cumsum(wnv)[lim]
    gamma = float(g_p.sum() + g_n.sum())
    return A0, A1, B0, B1, g_p, g_n, gamma


# ----------------------------------------------------------------------------
# bass program
# ----------------------------------------------------------------------------
def build(use_gelu=True):
    import concourse.bacc as bacc
    import concourse.tile as tile
    import concourse.mybir as mybir

    dt32 = mybir.dt.float32
    dt16 = mybir.dt.bfloat16
    GELU = (mybir.ActivationFunctionType.Gelu if use_gelu
            else mybir.ActivationFunctionType.Identity)

    nc = bacc.Bacc("TRN2", target_bir_lowering=False, debug=False,
                   num_devices=NCORES)
    mm = nc.tensor.matmul
    RG = [list(range(NCORES))]

    # ---- I/O ----
    xh_d = nc.dram_tensor("xh", [128, 5 * 256], dt16, kind="ExternalInput").ap()
    tp_d = nc.dram_tensor("tp", [128, 512], dt16, kind="ExternalInput").ap()
    aux_d = nc.dram_tensor("aux", [128, BPC * 3], dt16, kind="ExternalInput").ap()
    lm_d = nc.dram_tensor("lm", [5, 4], dt16, kind="ExternalInput").ap()
    xt_d = nc.dram_tensor("xt", [128, NBLK * 256], dt16, kind="ExternalInput").ap()
    w1_d = nc.dram_tensor("w1", [128, 64 * 512], dt16, kind="ExternalInput").ap()
    w2_d = nc.dram_tensor("w2", [128, KB * 512], dt16, kind="ExternalInput").ap()
    w3_d = nc.dram_tensor("w3", [128, KB * 512], dt16, kind="ExternalInput").ap()
    pj_d = nc.dram_tensor("pj", [128, 2 * 512], dt16, kind="ExternalInput").ap()
    b1_d = nc.dram_tensor("b1", [128, 4], dt32, kind="ExternalInput").ap()
    b2_d = nc.dram_tensor("b2", [128, 4], dt32, kind="ExternalInput").ap()
    b3_d = nc.dram_tensor("b3", [1, 512], dt16, kind="ExternalInput").ap()
    out_d = nc.dram_tensor("out", [Y0, HSH], dt32, kind="ExternalOutput").ap()

    with tile.TileContext(nc) as tc:
        with (
            tc.tile_pool(name="cst", bufs=1) as cst,
            tc.tile_pool(name="pn", bufs=3) as pnp,
            tc.tile_pool(name="wst", bufs=3) as wst,
            tc.tile_pool(name="psA", bufs=1, space="PSUM") as psA,
            tc.tile_pool(name="psB", bufs=2, space="PSUM") as psB,
            tc.tile_pool(name="dram", bufs=1, space="DRAM") as drp,
        ):
            # ---------- early small DMAs (stage-1 dependencies first) ----------
            xh_t = cst.tile([128, 5 * 256], dt16, tag="xh")
            nc.sync.dma_start(xh_t, xh_d)
            tp_t = cst.tile([128, 512], dt16, tag="tp")
            nc.scalar.dma_start(tp_t, tp_d)
            aux_t = cst.tile([128, BPC * 3], dt16, tag="aux")
            nc.scalar.dma_start(aux_t, aux_d)
            lm_t = cst.tile([5, 4], dt16, tag="lm")
            nc.scalar.dma_start(lm_t, lm_d)
            b1_t = cst.tile([128, 4], dt32, tag="b1")
            nc.scalar.dma_start(b1_t, b1_d)
            b2_t = cst.tile([128, 4], dt32, tag="b2")
            nc.scalar.dma_start(b2_t, b2_d)
            b3_t = cst.tile([1, 512], dt16, tag="b3")
            nc.scalar.dma_start(b3_t, b3_d)
            ones_t = cst.tile([128, 1], dt16, tag="ones")
            nc.vector.memset(ones_t, 1.0)
            onesr_t = cst.tile([1, 128], dt16, tag="onesr")
            nc.vector.memset(onesr_t, 1.0)

            # ---------- bulk streaming (fills the AllReduce shadow) ----------
            xt_t = cst.tile([128, NBLK * 256], dt16, tag="xt")
            for sp in range(2):
                w = NBLK * 256 // 2
                nc.sync.dma_start(xt_t[:, w * sp: w * (sp + 1)],
                                  xt_d[:, w * sp: w * (sp + 1)])
            # w1 streamed through a rotating pool: 4 G-half + 4 X-half chunks
            wg_tiles = []
            for ch in range(4):
                wt = wst.tile([128, 4096], dt16, tag="w", name=f"wg{ch}")
                nc.sync.dma_start(wt, w1_d[:, 4096 * ch: 4096 * (ch + 1)])
                wg_tiles.append(wt)
            wx_tiles = []
            for ch in range(4):
                wt = wst.tile([128, 4096], dt16, tag="w", name=f"wx{ch}")
                nc.scalar.dma_start(
                    wt, w1_d[:, 16384 + 4096 * ch: 16384 + 4096 * (ch + 1)])
                wx_tiles.append(wt)
            w2R = cst.tile([128, KB * 512], dt16, tag="w2R")
            for sp in range(4):
                w = KB * 512 // 4
                nc.sync.dma_start(w2R[:, w * sp: w * (sp + 1)],
                                  w2_d[:, w * sp: w * (sp + 1)])
            w3R = cst.tile([128, KB * 512], dt16, tag="w3R")
            for sp in range(4):
                w = KB * 512 // 4
                nc.scalar.dma_start(w3R[:, w * sp: w * (sp + 1)],
                                    w3_d[:, w * sp: w * (sp + 1)])
            pj_t = cst.tile([128, 2 * 512], dt16, tag="pj")
            nc.scalar.dma_start(pj_t, pj_d)

            # bounce buffers (DRAM)
            arA_i = drp.tile([261, NN], dt16, tag="arA_i")
            arA_o = drp.tile([261, NN], dt16, tag="arA_o", addr_space="Shared")
            g1_i = drp.tile([128, 1024], dt16, tag="g1_i", name="g1_i")
            g1_o = drp.tile([1024, 1024], dt16, tag="g1_o", name="g1_o",
                            addr_space="Shared")
            g2_i = drp.tile([128, 1024], dt16, tag="g2_i", name="g2_i")
            g2_o = drp.tile([1024, 1024], dt16, tag="g2_o", name="g2_o",
                            addr_space="Shared")

            # ---------- stage 1: cov partials over local time blocks ----------
            # u_ps[ic]: U.T chunk rows [128*ic, 128*ic+128); rac rows (r,a,c);
            # pq row = [p | q] (column sums of the P/N filter outputs)
            u_ps = [psA.tile([128, 256], dt32, tag=f"acc{ic}", name=f"u{ic}")
                    for ic in range(2)]
            rac_ps = psA.tile([3, 256], dt32, tag="acc2", name="rac_ps")
            pq_ps = psA.tile([1, 512], dt32, tag="acc3", name="pq_ps")
            for bl in range(BPC):
                xb = xh_t[:, 256 * bl: 256 * bl + 256]
                xb1 = xh_t[:, 256 * (bl + 1): 256 * (bl + 1) + 256]
                pn = pnp.tile([128, 512], dt16, tag="pn", name="pn")
                pt_ps = psB.tile([128, 256], dt32, tag="rot", name="pt_ps")
                mm(pt_ps, tp_t[:, 0:128], xb, start=True, stop=False)
                mm(pt_ps, tp_t[:, 128:256], xb1, start=False, stop=True)
                nc.vector.tensor_copy(pn[:, 0:256], pt_ps)
                nt_ps = psB.tile([128, 256], dt32, tag="rot", name="nt_ps")
                mm(nt_ps, tp_t[:, 256:384], xb, start=True, stop=False)
                mm(nt_ps, tp_t[:, 384:512], xb1, start=False, stop=True)
                nc.vector.tensor_copy(pn[:, 256:512], nt_ps)
                first, last = bl == 0, bl == BPC - 1
                for ic in range(2):
                    xbc = xh_t[:, 256 * bl + 128 * ic: 256 * bl + 128 * ic + 128]
                    mm(u_ps[ic], pn[:, 128 * ic:128 * ic + 128], xb,
                       start=first, stop=False)
                    mm(u_ps[ic], xbc, pn[:, 256:512], start=False, stop=last)
                mm(rac_ps, aux_t[:, 3 * bl:3 * bl + 3], xb,
                   start=first, stop=last)
                mm(pq_ps, ones_t, pn[:, 0:512], start=first, stop=last)

            # pack (bf16) + stage + AllReduce (doubles as the rendezvous)
            stgs = []
            for ic in range(2):
                stg = cst.tile([128, 256], dt16, tag=f"stg{ic}", name=f"stg{ic}")
                nc.vector.tensor_scalar_mul(stg, u_ps[ic], 1.0 / DD)
                stgs.append(stg)
            vrac = cst.tile([3, 256], dt16, tag="vrac", name="vrac")
            nc.vector.tensor_copy(vrac, rac_ps)
            vpq = cst.tile([1, 512], dt16, tag="vpq", name="vpq")
            nc.vector.tensor_copy(vpq, pq_ps)
            nc.scalar.dma_start(arA_i[0:128, :], stgs[0])
            nc.scalar.dma_start(arA_i[128:256, :], stgs[1])
            nc.scalar.dma_start(arA_i[256:259, :], vrac)
            nc.scalar.dma_start(
                arA_i[259:261, :].rearrange("(b p) n -> b p n", p=1)
                .transpose([1, 0, 2]),
                vpq.rearrange("p (b n) -> p b n", b=2))
            nc.gpsimd.collective_compute(
                "AllReduce", mybir.AluOpType.add, replica_groups=RG,
                ins=[arA_i.opt()], outs=[arA_o.opt()])

            # ---------- G^T = X @ W1c (cov-independent, in AR shadow) ----------
            g_ps = [psA.tile([128, 512], dt32, tag=f"acc{4 + ib}", name=f"g_{ib}")
                    for ib in range(2)]
            for ch in range(4):
                for kl in range(8):
                    k = 8 * ch + kl
                    for ib in range(2):
                        mm(g_ps[ib],
                           xt_t[:, 256 * k + 128 * ib: 256 * k + 128 * ib + 128],
                           wg_tiles[ch][:, 512 * kl: 512 * kl + 512],
                           start=(k == 0), stop=(k == NBLK - 1))
            gT = cst.tile([128, 2 * 512], dt16, tag="gT")
            for ib in range(2):
                nc.vector.tensor_copy(gT[:, 512 * ib:512 * ib + 512], g_ps[ib])

            # ---------- fc1 X-half (also in AR shadow) ----------
            f1_ps = [psA.tile([128, 256], dt32, tag=f"acc{hh}", name=f"f1_{hh}")
                     for hh in range(4)]
            for ch in range(4):
                for kl in range(8):
                    k = 8 * ch + kl
                    for hh in range(4):
                        mm(f1_ps[hh],
                           wx_tiles[ch][:, 512 * kl + 128 * hh: 512 * kl + 128 * hh + 128],
                           xt_t[:, 256 * k:256 * k + 256],
                           start=(k == 0), stop=False)

            # ---------- post-AR: m/alpha/beta + cov.T ----------
            rows = cst.tile([5, 256], dt16, tag="rows", name="rows")
            nc.sync.dma_start(rows, arA_o[256:261, :])
            ured = cst.tile([128, 512], dt16, tag="ured", name="ured")
            nc.sync.dma_start(ured[:, 0:256], arA_o[0:128, :])
            nc.scalar.dma_start(ured[:, 256:512], arA_o[128:256, :])
            # m/alpha/beta as three 256-col segments of one partition-0 row
            ma_ps = psB.tile([1, 512], dt32, tag="rot", name="ma_ps")
            for s in range(2):
                mm(ma_ps[0:1, 256 * s: 256 * s + 256], lm_t[:, s:s + 1], rows,
                   start=True, stop=True)
            be_ps = psB.tile([1, 256], dt32, tag="rot", name="be_ps")
            mm(be_ps, lm_t[:, 2:3], rows, start=True, stop=True)
            mab = cst.tile([1, 3 * 256], dt16, tag="mab", name="mab")
            nc.vector.tensor_copy(mab[0:1, 0:512], ma_ps)
            nc.vector.tensor_copy(mab[0:1, 512:768], be_ps)
            covt = cst.tile([128, 2 * 256], dt16, tag="covt")
            for ic in range(2):
                corr = psB.tile([128, 256], dt32, tag="rot", name="corr")
                mm(corr, mab[0:1, 128 * ic:128 * ic + 128], mab[0:1, 256:512],
                   start=True, stop=False)
                mm(corr, mab[0:1, 512 + 128 * ic:512 + 128 * ic + 128],
                   mab[0:1, 0:256], start=False, stop=True)
                nc.vector.tensor_sub(covt[:, 256 * ic:256 * ic + 256],
                                     ured[:, 256 * ic:256 * ic + 256], corr)

            # ---------- fc1 cov contribution + gelu + chunked AllGather ----------
            a1loc = cst.tile([128, 4 * 256], dt16, tag="a1loc")
            for hh in range(4):
                for ib in range(2):
                    mm(f1_ps[hh],
                       gT[:, 512 * ib + 128 * hh: 512 * ib + 128 * hh + 128],
                       covt[:, 256 * ib:256 * ib + 256],
                       start=False, stop=(ib == 1))
                nc.scalar.activation(a1loc[:, 256 * hh:256 * hh + 256],
                                     f1_ps[hh], GELU, bias=b1_t[:, hh:hh + 1])
            nc.scalar.dma_start(g1_i, a1loc)
            nc.gpsimd.collective_compute(
                "AllGather", mybir.AluOpType.bypass, replica_groups=RG,
                ins=[g1_i.opt()], outs=[g1_o.opt()])

            # ---------- fc2 (chunked over the two gathers) ----------
            a1f = [cst.tile([128, 16 * 256], dt16, tag=f"a1f{c}", name=f"a1f{c}")
                   for c in range(2)]
            src1 = (g1_o.rearrange("(r p) n -> r p n", p=128)
                    .transpose([1, 0, 2]))           # [128, 8, 1024]
            for r in range(8):
                dst = a1f[r // 4].rearrange("p (r n) -> p r n", r=4)
                eng = nc.sync if r % 2 == 0 else nc.scalar
                eng.dma_start(dst[:, r % 4:r % 4 + 1, :], src1[:, r:r + 1, :])
            f2_ps = [psA.tile([128, 256], dt32, tag=f"acc{hh}", name=f"f2_{hh}")
                     for hh in range(4)]
            for j in range(16):                      # first half, all hh
                for hh in range(4):
                    mm(f2_ps[hh],
                       w2R[:, 512 * j + 128 * hh: 512 * j + 128 * hh + 128],
                       a1f[0][:, 256 * j:256 * j + 256],
                       start=(j == 0), stop=False)
            a2loc = cst.tile([128, 4 * 256], dt16, tag="a2loc")
            for hh in range(4):                      # second half hh-major
                for j in range(16):
                    mm(f2_ps[hh],
                       w2R[:, 512 * (16 + j) + 128 * hh: 512 * (16 + j) + 128 * hh + 128],
                       a1f[1][:, 256 * j:256 * j + 256],
                       start=False, stop=(j == 15))
                nc.scalar.activation(a2loc[:, 256 * hh:256 * hh + 256],
                                     f2_ps[hh], GELU, bias=b2_t[:, hh:hh + 1])
            nc.scalar.dma_start(g2_i, a2loc)
            nc.gpsimd.collective_compute(
                "AllGather", mybir.AluOpType.bypass, replica_groups=RG,
                ins=[g2_i.opt()], outs=[g2_o.opt()])

            # ---------- fc3 (batch-major out, chunked) ----------
            a2f = [cst.tile([128, 16 * 256], dt16, tag=f"a2f{c}", name=f"a2f{c}")
                   for c in range(2)]
            src2 = (g2_o.rearrange("(r p) n -> r p n", p=128)
                    .transpose([1, 0, 2]))
            for r in range(8):
                dst = a2f[r // 4].rearrange("p (r n) -> p r n", r=4)
                eng = nc.sync if r % 2 == 0 else nc.scalar
                eng.dma_start(dst[:, r % 4:r % 4 + 1, :], src2[:, r:r + 1, :])
            f3_ps = [psA.tile([128, 512], dt32, tag=f"acc{4 + ii}", name=f"f3_{ii}")
                     for ii in range(2)]
            o3_t = cst.tile([128, 2 * 512], dt16, tag="o3")
            for c in range(2):
                for j in range(16):
                    for ii in range(2):
                        mm(f3_ps[ii],
                           a2f[c][:, 256 * j + 128 * ii: 256 * j + 128 * ii + 128],
                           w3R[:, 512 * (16 * c + j): 512 * (16 * c + j) + 512],
                           start=(c == 0 and j == 0), stop=False)
            for ii in range(2):
                mm(f3_ps[ii], onesr_t, b3_t, start=False, stop=True)

            # ---------- proj (interleaved with o3 evacuation) ----------
            po = [psA.tile([128, 512], dt32, tag=f"acc{pp}", name=f"po{pp}")
                  for pp in range(4)]
            for ii in range(2):
                nc.vector.tensor_copy(o3_t[:, 512 * ii:512 * ii + 512], f3_ps[ii])
                for pp in range(4):
                    mm(po[pp],
                       pj_t[:, 512 * ii + 128 * pp: 512 * ii + 128 * pp + 128],
                       o3_t[:, 512 * ii:512 * ii + 512],
                       start=(ii == 0), stop=(ii == 1))
            for pp in range(4):
                osb = cst.tile([128, 512], dt32, tag=f"osb{pp}", name=f"osb{pp}")
                nc.vector.tensor_copy(osb, po[pp])
                nc.sync.dma_start(out_d[128 * pp:128 * pp + 128, :], osb)

    nc.compile()
    return nc


# ----------------------------------------------------------------------------
# host-side sharding / packing
# ----------------------------------------------------------------------------
def prep_in_maps(inputs):
    X = np.asarray(inputs["X"], np.float32)
    A0, A1, B0, B1, g_p, g_n, gamma = _filters(inputs)

    XT = np.ascontiguousarray(X.T)                      # [D, N]
    xt = XT.reshape(NBLK, 128, NN).transpose(1, 0, 2).reshape(128, NBLK * 256)
    xt = xt.astype(bf16)
    tp = np.concatenate([A0, A1, B0, B1], axis=1).astype(bf16)
    pjT = np.asarray(inputs["proj"], np.float64).T      # [256, 512]
    pj = pjT.reshape(2, 128, 512).transpose(1, 0, 2).reshape(128, 1024).astype(bf16)

    lm = np.zeros((5, 4), np.float64)
    lm[0, 0] = 1.0 / DD                 # m  <- r
    lm[0, 1] = -gamma / DD**2           # al <- r
    lm[1, 1] = 1.0 / DD                 # al <- a
    lm[4, 1] = 1.0 / DD                 # al <- q
    lm[2, 2] = 1.0 / DD                 # be <- c
    lm[3, 2] = 1.0 / DD                 # be <- p
    lm = lm.astype(bf16)

    f64 = lambda k: np.asarray(inputs[k], np.float64)
    fc_wT = {1: f64("fc1_w").T, 2: f64("fc2_w").T, 3: f64("fc3_w").T}

    XTz = np.concatenate([XT, np.zeros((128, NN), np.float32)], axis=0)

    in_maps = []
    for c in range(NCORES):
        xh = np.zeros((128, 5 * 256), np.float32)
        for bl in range(5):
            gb = 4 * c + bl
            xh[:, 256 * bl: 256 * bl + 256] = XTz[128 * gb:128 * gb + 128]
        aux = np.zeros((128, BPC * 3), np.float32)
        for bl in range(BPC):
            gb = 4 * c + bl
            aux[:, 3 * bl + 0] = 1.0
            aux[:, 3 * bl + 1] = g_p[128 * gb:128 * gb + 128]
            aux[:, 3 * bl + 2] = g_n[128 * gb:128 * gb + 128]
        hs = slice(HSH * c, HSH * (c + 1))
        # w1: G-half (cov rows 4096..8191) first, then X-half
        w1rows = np.concatenate([fc_wT[1][DD:, hs], fc_wT[1][:DD, hs]], axis=0)
        w1 = w1rows.reshape(64, 128, HSH).transpose(1, 0, 2) \
            .reshape(128, 64 * HSH).astype(bf16)
        w2 = fc_wT[2][:, hs].reshape(KB, 128, HSH) \
            .transpose(1, 0, 2).reshape(128, KB * HSH).astype(bf16)
        w3 = fc_wT[3][:, hs].reshape(KB, 128, HSH) \
            .transpose(1, 0, 2).reshape(128, KB * HSH).astype(bf16)
        b1 = f64("fc1_b")[hs].reshape(4, 128).T.astype(np.float32)
        b2 = f64("fc2_b")[hs].reshape(4, 128).T.astype(np.float32)
        b3 = f64("fc3_b")[hs].reshape(1, HSH).astype(bf16)
        in_maps.append({
            "xt": xt, "xh": xh.astype(bf16), "tp": tp, "lm": lm,
            "aux": aux.astype(bf16), "w1": w1, "w2": w2, "w3": w3,
            "pj": pj, "b1": b1, "b2": b2, "b3": b3,
        })
    return in_maps


def run(inputs, trace=False, **kw):
    in_maps = prep_in_maps(inputs)
    if "nc" not in _CACHE:
        _CACHE["nc"] = build()
    nc = _CACHE["nc"]
    from concourse import bass_utils
    res = bass_utils.run_bass_kernel_spmd(nc, in_maps,
                                          core_ids=list(range(NCORES)),
                                          trace=trace, **kw)
    out = np.concatenate([res.results[c]["out"] for c in range(NCORES)], axis=1)
    return out.astype(np.float32), res


def kernel(**inputs) -> np.ndarray:
    out, _ = run(inputs)
    return out


if __name__ == "__main__":
    data = np.load("inputs.npz")
    inputs = {k: data[k] for k in data.files}
    expected = np.load("expected.npy")
    out = kernel(**inputs)
    scale = np.abs(expected).max()
    err = np.abs(out - expected).max() / scale
    print(f"Relative error: {err:.3e}")


# revision 12
# speedup vs baseline: 1.1410x; 1.0245x over previous
"""Trainium2 Bass kernel for nn_Model1 (lag-weighted long-run covariance + MLP).

Math: the 129-lag weighted covariance collapses algebraically:
    sum_l w_l * (Xc @ Y_l.T) = Xc @ (sum_l w_l Y_l).T
so cov*d = Xc @ P.T + N @ Xc.T with P, N two 65-tap causal FIR filters of Xc.
Centering is pushed through the filters as rank-1 corrections so all GEMMs
run on UNCENTERED X:
    cov.T = U.T/d - m (x) alpha - beta (x) m
with U = X@P0.T + N0@X.T (P0,N0 = filters of raw X), m = row means, and
alpha/beta linear in 5 reduction vectors (r,a,c,p,q) that ride along as
extra rows of the U AllReduce.  (m,alpha,beta) = L @ [r;a;c;p;q] for a
constant 5x3 matrix L shipped as a tiny input.

Distribution (8 cores):
  - cov stage: shard time axis (512 cols/core), one bf16 AllReduce of
    [261,256], triggered as early as possible (it doubles as the inter-core
    rendezvous, absorbing launch skew).
  - MLP: tensor-parallel over hidden (512/core).  The activation AllGathers
    between fc1->fc2 and fc2->fc3 are split in 2 chunks each so the second
    chunk's transfer overlaps the first chunk's matmuls.
  - fc3 emits batch-major so proj shards the output columns; final gather is
    a host-side concat.
All heavy GEMMs use bf16 operands with fp32 PSUM accumulation.  All weights
stream during the AllReduce window so post-AR compute is never DMA-gated.
"""
import math
import numpy as np
import ml_dtypes

NCORES = 8
Q = 64
NN = 256          # n (batch/rows of X)
DD = 4096         # d (time axis)
HID = 4096
Y0 = 512
HSH = HID // NCORES    # 512 hidden shard per core
NBLK = DD // 128       # 32 time blocks
BPC = NBLK // NCORES   # 4 blocks per core
KB = HID // 128        # 32 fc2/fc3 contraction blocks

bf16 = ml_dtypes.bfloat16

_CACHE = {}


# ----------------------------------------------------------------------------
# host-side weight-only precompute
# ----------------------------------------------------------------------------
def _erf(x):
    return np.vectorize(math.erf, otypes=[np.float64])(x)


def _gelu64(x):
    return 0.5 * x * (1.0 + _erf(x / np.sqrt(2.0)))


def _filters(inputs):
    f64 = lambda k: np.asarray(inputs[k], np.float64)
    lags = np.arange(-Q, Q + 1, dtype=np.float64)[:, None]
    h = _gelu64(lags @ f64("wn_w1") + f64("wn_b1"))
    w = (h @ f64("wn_w2") + f64("wn_b2"))[:, 0]
    wp = w[Q:]                                    # l = 0..Q
    wnv = np.concatenate([[0.0], w[:Q][::-1]])    # wnv[l] = w[Q-l], l=1..Q
    v = np.arange(128)[:, None]
    u = np.arange(128)[None, :]
    dvu = v - u
    d2 = dvu + 128
    A0 = np.where((dvu >= 0) & (dvu <= Q), wp[np.clip(dvu, 0, Q)], 0.0)
    A1 = np.where((d2 >= 0) & (d2 <= Q), wp[np.clip(d2, 0, Q)], 0.0)
    B0 = np.where((dvu >= 1) & (dvu <= Q), wnv[np.clip(dvu, 0, Q)], 0.0)
    B1 = np.where((d2 >= 1) & (d2 <= Q), wnv[np.clip(d2, 0, Q)], 0.0)
    t = np.arange(DD)
    lim = np.minimum(Q, DD - 1 - t)
    g_p = np.cumsum(wp)[lim]
    g_n = np.cumsum(wnv)[lim]
    gamma = float(g_p.sum() + g_n.sum())
    return A0, A1, B0, B1, g_p, g_n, gamma


# ----------------------------------------------------------------------------
# bass program
# ----------------------------------------------------------------------------
def build(use_gelu=True):
    import concourse.bacc as bacc
    import concourse.tile as tile
    import concourse.mybir as mybir

    dt32 = mybir.dt.float32
    dt16 = mybir.dt.bfloat16
    GELU = (mybir.ActivationFunctionType.Gelu if use_gelu
            else mybir.ActivationFunctionType.Identity)

    nc = bacc.Bacc("TRN2", target_bir_lowering=False, debug=False,
                   num_devices=NCORES)
    mm = nc.tensor.matmul
    RG = [list(range(NCORES))]

    # ---- I/O ----
    xh_d = nc.dram_tensor("xh", [128, 5 * 256], dt16, kind="ExternalInput").ap()
    tp_d = nc.dram_tensor("tp", [128, 512], dt16, kind="ExternalInput").ap()
    aux_d = nc.dram_tensor("aux", [128, BPC * 3], dt16, kind="ExternalInput").ap()
    lm_d = nc.dram_tensor("lm", [5, 4], dt16, kind="ExternalInput").ap()
    xt_d = nc.dram_tensor("xt", [128, NBLK * 256], dt16, kind="ExternalInput").ap()
    w1_d = nc.dram_tensor("w1", [128, 64 * 512], dt16, kind="ExternalInput").ap()
    w2_d = nc.dram_tensor("w2", [128, KB * 512], dt16, kind="ExternalInput").ap()
    w3_d = nc.dram_tensor("w3", [128, KB * 512], dt16, kind="ExternalInput").ap()
    pj_d = nc.dram_tensor("pj", [128, 2 * 512], dt16, kind="ExternalInput").ap()
    b1_d = nc.dram_tensor("b1", [128, 4], dt32, kind="ExternalInput").ap()
    b2_d = nc.dram_tensor("b2", [128, 4], dt32, kind="ExternalInput").ap()
    b3_d = nc.dram_tensor("b3", [1, 512], dt16, kind="ExternalInput").ap()
    out_d = nc.dram_tensor("out", [Y0, HSH], dt32, kind="ExternalOutput").ap()

    with tile.TileContext(nc) as tc:
        with (
            tc.tile_pool(name="cst", bufs=1) as cst,
            tc.tile_pool(name="pn", bufs=3) as pnp,
            tc.tile_pool(name="wst", bufs=3) as wst,
            tc.tile_pool(name="psA", bufs=1, space="PSUM") as psA,
            tc.tile_pool(name="psB", bufs=2, space="PSUM") as psB,
            tc.tile_pool(name="dram", bufs=1, space="DRAM") as drp,
        ):
            # ---------- early small DMAs (stage-1 dependencies first) ----------
            xh_t = cst.tile([128, 5 * 256], dt16, tag="xh")
            nc.sync.dma_start(xh_t, xh_d)
            tp_t = cst.tile([128, 512], dt16, tag="tp")
            nc.scalar.dma_start(tp_t, tp_d)
            aux_t = cst.tile([128, BPC * 3], dt16, tag="aux")
            nc.scalar.dma_start(aux_t, aux_d)
            lm_t = cst.tile([5, 4], dt16, tag="lm")
            nc.scalar.dma_start(lm_t, lm_d)
            b1_t = cst.tile([128, 4], dt32, tag="b1")
            nc.scalar.dma_start(b1_t, b1_d)
            b2_t = cst.tile([128, 4], dt32, tag="b2")
            nc.scalar.dma_start(b2_t, b2_d)
            b3_t = cst.tile([1, 512], dt16, tag="b3")
            nc.scalar.dma_start(b3_t, b3_d)
            ones_t = cst.tile([128, 1], dt16, tag="ones")
            nc.vector.memset(ones_t, 1.0)
            onesr_t = cst.tile([1, 128], dt16, tag="onesr")
            nc.vector.memset(onesr_t, 1.0)

            # ---------- bulk streaming (fills the AllReduce shadow) ----------
            xt_t = cst.tile([128, NBLK * 256], dt16, tag="xt")
            for sp in range(2):
                w = NBLK * 256 // 2
                nc.sync.dma_start(xt_t[:, w * sp: w * (sp + 1)],
                                  xt_d[:, w * sp: w * (sp + 1)])
            # w1 streamed through a rotating pool: 4 G-half + 4 X-half chunks
            wg_tiles = []
            for ch in range(4):
                wt = wst.tile([128, 4096], dt16, tag="w", name=f"wg{ch}")
                nc.sync.dma_start(wt, w1_d[:, 4096 * ch: 4096 * (ch + 1)])
                wg_tiles.append(wt)
            wx_tiles = []
            for ch in range(4):
                wt = wst.tile([128, 4096], dt16, tag="w", name=f"wx{ch}")
                nc.scalar.dma_start(
                    wt, w1_d[:, 16384 + 4096 * ch: 16384 + 4096 * (ch + 1)])
                wx_tiles.append(wt)
            w2R = cst.tile([128, KB * 512], dt16, tag="w2R")
            for sp in range(4):
                w = KB * 512 // 4
                nc.sync.dma_start(w2R[:, w * sp: w * (sp + 1)],
                                  w2_d[:, w * sp: w * (sp + 1)])
            w3R = cst.tile([128, KB * 512], dt16, tag="w3R")
            for sp in range(4):
                w = KB * 512 // 4
                nc.scalar.dma_start(w3R[:, w * sp: w * (sp + 1)],
                                    w3_d[:, w * sp: w * (sp + 1)])
            pj_t = cst.tile([128, 2 * 512], dt16, tag="pj")
            nc.scalar.dma_start(pj_t, pj_d)

            # bounce buffers (DRAM)
            arA_i = drp.tile([261, NN], dt16, tag="arA_i")
            arA_o = drp.tile([261, NN], dt16, tag="arA_o", addr_space="Shared")
            g1_i = [drp.tile([128, 512], dt16, tag=f"g1_i{nh}", name=f"g1_i{nh}")
                    for nh in range(2)]
            g1_o = [drp.tile([1024, 512], dt16, tag=f"g1_o{nh}", name=f"g1_o{nh}",
                             addr_space="Shared") for nh in range(2)]
            g2_i = [drp.tile([128, 512], dt16, tag=f"g2_i{nh}", name=f"g2_i{nh}")
                    for nh in range(2)]
            g2_o = [drp.tile([1024, 512], dt16, tag=f"g2_o{nh}", name=f"g2_o{nh}",
                             addr_space="Shared") for nh in range(2)]

            # ---------- stage 1: cov partials over local time blocks ----------
            # u_ps[ic]: U.T chunk rows [128*ic, 128*ic+128); rac rows (r,a,c);
            # pq row = [p | q] (column sums of the P/N filter outputs)
            u_ps = [psA.tile([128, 256], dt32, tag=f"acc{ic}", name=f"u{ic}")
                    for ic in range(2)]
            rac_ps = psA.tile([3, 256], dt32, tag="acc2", name="rac_ps")
            pq_ps = psA.tile([1, 512], dt32, tag="acc3", name="pq_ps")
            for bl in range(BPC):
                xb = xh_t[:, 256 * bl: 256 * bl + 256]
                xb1 = xh_t[:, 256 * (bl + 1): 256 * (bl + 1) + 256]
                pn = pnp.tile([128, 512], dt16, tag="pn", name="pn")
                pt_ps = psB.tile([128, 256], dt32, tag="rot", name="pt_ps")
                mm(pt_ps, tp_t[:, 0:128], xb, start=True, stop=False)
                mm(pt_ps, tp_t[:, 128:256], xb1, start=False, stop=True)
                nc.vector.tensor_copy(pn[:, 0:256], pt_ps)
                nt_ps = psB.tile([128, 256], dt32, tag="rot", name="nt_ps")
                mm(nt_ps, tp_t[:, 256:384], xb, start=True, stop=False)
                mm(nt_ps, tp_t[:, 384:512], xb1, start=False, stop=True)
                nc.vector.tensor_copy(pn[:, 256:512], nt_ps)
                first, last = bl == 0, bl == BPC - 1
                for ic in range(2):
                    xbc = xh_t[:, 256 * bl + 128 * ic: 256 * bl + 128 * ic + 128]
                    mm(u_ps[ic], pn[:, 128 * ic:128 * ic + 128], xb,
                       start=first, stop=False)
                    mm(u_ps[ic], xbc, pn[:, 256:512], start=False, stop=last)
                mm(rac_ps, aux_t[:, 3 * bl:3 * bl + 3], xb,
                   start=first, stop=last)
                mm(pq_ps, ones_t, pn[:, 0:512], start=first, stop=last)

            # pack (bf16) + stage + AllReduce (doubles as the rendezvous)
            stgs = []
            for ic in range(2):
                stg = cst.tile([128, 256], dt16, tag=f"stg{ic}", name=f"stg{ic}")
                nc.vector.tensor_scalar_mul(stg, u_ps[ic], 1.0 / DD)
                stgs.append(stg)
            vrac = cst.tile([3, 256], dt16, tag="vrac", name="vrac")
            nc.vector.tensor_copy(vrac, rac_ps)
            vpq = cst.tile([1, 512], dt16, tag="vpq", name="vpq")
            nc.vector.tensor_copy(vpq, pq_ps)
            nc.scalar.dma_start(arA_i[0:128, :], stgs[0])
            nc.scalar.dma_start(arA_i[128:256, :], stgs[1])
            nc.scalar.dma_start(arA_i[256:259, :], vrac)
            nc.scalar.dma_start(
                arA_i[259:261, :].rearrange("(b p) n -> b p n", p=1)
                .transpose([1, 0, 2]),
                vpq.rearrange("p (b n) -> p b n", b=2))
            nc.gpsimd.collective_compute(
                "AllReduce", mybir.AluOpType.add, replica_groups=RG,
                ins=[arA_i.opt()], outs=[arA_o.opt()])

            # ---------- G^T = X @ W1c (cov-independent, in AR shadow) ----------
            g_ps = [psA.tile([128, 512], dt32, tag=f"acc{4 + ib}", name=f"g_{ib}")
                    for ib in range(2)]
            for ch in range(4):
                for kl in range(8):
                    k = 8 * ch + kl
                    for ib in range(2):
                        mm(g_ps[ib],
                           xt_t[:, 256 * k + 128 * ib: 256 * k + 128 * ib + 128],
                           wg_tiles[ch][:, 512 * kl: 512 * kl + 512],
                           start=(k == 0), stop=(k == NBLK - 1))
            gT = cst.tile([128, 2 * 512], dt16, tag="gT")
            for ib in range(2):
                nc.vector.tensor_copy(gT[:, 512 * ib:512 * ib + 512], g_ps[ib])

            # ---------- fc1 X-half (also in AR shadow) ----------
            f1_ps = [psA.tile([128, 256], dt32, tag=f"acc{hh}", name=f"f1_{hh}")
                     for hh in range(4)]
            for ch in range(4):
                for kl in range(8):
                    k = 8 * ch + kl
                    for hh in range(4):
                        mm(f1_ps[hh],
                           wx_tiles[ch][:, 512 * kl + 128 * hh: 512 * kl + 128 * hh + 128],
                           xt_t[:, 256 * k:256 * k + 256],
                           start=(k == 0), stop=False)

            # ---------- post-AR: m/alpha/beta + cov.T ----------
            rows = cst.tile([5, 256], dt16, tag="rows", name="rows")
            nc.sync.dma_start(rows, arA_o[256:261, :])
            ured = cst.tile([128, 512], dt16, tag="ured", name="ured")
            nc.sync.dma_start(ured[:, 0:256], arA_o[0:128, :])
            nc.scalar.dma_start(ured[:, 256:512], arA_o[128:256, :])
            # m/alpha/beta as three 256-col segments of one partition-0 row
            ma_ps = psB.tile([1, 512], dt32, tag="rot", name="ma_ps")
            for s in range(2):
                mm(ma_ps[0:1, 256 * s: 256 * s + 256], lm_t[:, s:s + 1], rows,
                   start=True, stop=True)
            be_ps = psB.tile([1, 256], dt32, tag="rot", name="be_ps")
            mm(be_ps, lm_t[:, 2:3], rows, start=True, stop=True)
            mab = cst.tile([1, 3 * 256], dt16, tag="mab", name="mab")
            nc.vector.tensor_copy(mab[0:1, 0:512], ma_ps)
            nc.vector.tensor_copy(mab[0:1, 512:768], be_ps)
            covt = cst.tile([128, 2 * 256], dt16, tag="covt")
            for ic in range(2):
                corr = psB.tile([128, 256], dt32, tag="rot", name="corr")
                mm(corr, mab[0:1, 128 * ic:128 * ic + 128], mab[0:1, 256:512],
                   start=True, stop=False)
                mm(corr, mab[0:1, 512 + 128 * ic:512 + 128 * ic + 128],
                   mab[0:1, 0:256], start=False, stop=True)
                nc.vector.tensor_sub(covt[:, 256 * ic:256 * ic + 256],
                                     ured[:, 256 * ic:256 * ic + 256], corr)

            # ---------- fc1 cov contribution + gelu (batch-half pipelined) ----
            # a1loc layout: [128 h, 2 nh * 4 hh * 128 n]  (n-half major)
            a1loc = cst.tile([128, 4 * 256], dt16, tag="a1loc")
            for hh in range(4):
                for ib in range(2):
                    mm(f1_ps[hh],
                       gT[:, 512 * ib + 128 * hh: 512 * ib + 128 * hh + 128],
                       covt[:, 256 * ib:256 * ib + 256],
                       start=False, stop=(ib == 1))
                for nh in range(2):
                    nc.scalar.activation(
                        a1loc[:, 512 * nh + 128 * hh: 512 * nh + 128 * hh + 128],
                        f1_ps[hh][:, 128 * nh:128 * nh + 128],
                        GELU, bias=b1_t[:, hh:hh + 1])
            for nh in range(2):
                nc.scalar.dma_start(g1_i[nh], a1loc[:, 512 * nh:512 * nh + 512])
                nc.gpsimd.collective_compute(
                    "AllGather", mybir.AluOpType.bypass, replica_groups=RG,
                    ins=[g1_i[nh].opt()], outs=[g1_o[nh].opt()])

            # ---------- fc2 (per batch-half, per-rank pipelined loads) ----------
            a1f = [cst.tile([128, 8 * 512], dt16, tag=f"a1f{nh}", name=f"a1f{nh}")
                   for nh in range(2)]
            a2loc = cst.tile([128, 4 * 256], dt16, tag="a2loc")
            f2h = [[psA.tile([128, 128], dt32, tag=f"acc{hh}", name=f"f2_{nh}{hh}")
                    for hh in range(4)] for nh in range(2)]
            for nh in range(2):
                srcg = (g1_o[nh].rearrange("(r p) w -> r p w", p=128)
                        .transpose([1, 0, 2]))
                dst = a1f[nh].rearrange("p (r w) -> p r w", r=8)
                for r in range(8):
                    eng = nc.sync if r % 2 == 0 else nc.scalar
                    eng.dma_start(dst[:, r:r + 1, :], srcg[:, r:r + 1, :])
                for b in range(KB):
                    r, hh = b // 4, b % 4
                    for h2 in range(4):
                        mm(f2h[nh][h2],
                           w2R[:, 512 * b + 128 * h2: 512 * b + 128 * h2 + 128],
                           a1f[nh][:, 512 * r + 128 * hh: 512 * r + 128 * hh + 128],
                           start=(b == 0), stop=(b == KB - 1))
                for h2 in range(4):
                    nc.scalar.activation(
                        a2loc[:, 512 * nh + 128 * h2: 512 * nh + 128 * h2 + 128],
                        f2h[nh][h2], GELU, bias=b2_t[:, h2:h2 + 1])
                nc.scalar.dma_start(g2_i[nh], a2loc[:, 512 * nh:512 * nh + 512])
                nc.gpsimd.collective_compute(
                    "AllGather", mybir.AluOpType.bypass, replica_groups=RG,
                    ins=[g2_i[nh].opt()], outs=[g2_o[nh].opt()])

            # ---------- fc3 (batch-major out, per batch-half) ----------
            a2f = [cst.tile([128, 8 * 512], dt16, tag=f"a2f{nh}", name=f"a2f{nh}")
                   for nh in range(2)]
            f3_ps = [psA.tile([128, 512], dt32, tag=f"acc{4 + nh}", name=f"f3_{nh}")
                     for nh in range(2)]
            o3_t = cst.tile([128, 2 * 512], dt16, tag="o3")
            po = [psA.tile([128, 512], dt32, tag=f"acc{pp}", name=f"po{pp}")
                  for pp in range(4)]
            for nh in range(2):
                srcg = (g2_o[nh].rearrange("(r p) w -> r p w", p=128)
                        .transpose([1, 0, 2]))
                dst = a2f[nh].rearrange("p (r w) -> p r w", r=8)
                for r in range(8):
                    eng = nc.sync if r % 2 == 0 else nc.scalar
                    eng.dma_start(dst[:, r:r + 1, :], srcg[:, r:r + 1, :])
                for b in range(KB):
                    r, hh = b // 4, b % 4
                    mm(f3_ps[nh],
                       a2f[nh][:, 512 * r + 128 * hh: 512 * r + 128 * hh + 128],
                       w3R[:, 512 * b: 512 * b + 512],
                       start=(b == 0), stop=False)
                mm(f3_ps[nh], onesr_t, b3_t, start=False, stop=True)
                # proj contribution of this batch half
                nc.vector.tensor_copy(o3_t[:, 512 * nh:512 * nh + 512], f3_ps[nh])
                for pp in range(4):
                    mm(po[pp],
                       pj_t[:, 512 * nh + 128 * pp: 512 * nh + 128 * pp + 128],
                       o3_t[:, 512 * nh:512 * nh + 512],
                       start=(nh == 0), stop=(nh == 1))
            for pp in range(4):
                osb = cst.tile([128, 512], dt32, tag=f"osb{pp}", name=f"osb{pp}")
                nc.vector.tensor_copy(osb, po[pp])
                nc.sync.dma_start(out_d[128 * pp:128 * pp + 128, :], osb)

    nc.compile()
    return nc


# ----------------------------------------------------------------------------
# host-side sharding / packing
# ----------------------------------------------------------------------------
def prep_in_maps(inputs):
    X = np.asarray(inputs["X"], np.float32)
    A0, A1, B0, B1, g_p, g_n, gamma = _filters(inputs)

    XT = np.ascontiguousarray(X.T)                      # [D, N]
    xt = XT.reshape(NBLK, 128, NN).transpose(1, 0, 2).reshape(128, NBLK * 256)
    xt = xt.astype(bf16)
    tp = np.concatenate([A0, A1, B0, B1], axis=1).astype(bf16)
    pjT = np.asarray(inputs["proj"], np.float64).T      # [256, 512]
    pj = pjT.reshape(2, 128, 512).transpose(1, 0, 2).reshape(128, 1024).astype(bf16)

    lm = np.zeros((5, 4), np.float64)
    lm[0, 0] = 1.0 / DD                 # m  <- r
    lm[0, 1] = -gamma / DD**2           # al <- r
    lm[1, 1] = 1.0 / DD                 # al <- a
    lm[4, 1] = 1.0 / DD                 # al <- q
    lm[2, 2] = 1.0 / DD                 # be <- c
    lm[3, 2] = 1.0 / DD                 # be <- p
    lm = lm.astype(bf16)

    f64 = lambda k: np.asarray(inputs[k], np.float64)
    fc_wT = {1: f64("fc1_w").T, 2: f64("fc2_w").T, 3: f64("fc3_w").T}

    XTz = np.concatenate([XT, np.zeros((128, NN), np.float32)], axis=0)

    in_maps = []
    for c in range(NCORES):
        xh = np.zeros((128, 5 * 256), np.float32)
        for bl in range(5):
            gb = 4 * c + bl
            xh[:, 256 * bl: 256 * bl + 256] = XTz[128 * gb:128 * gb + 128]
        aux = np.zeros((128, BPC * 3), np.float32)
        for bl in range(BPC):
            gb = 4 * c + bl
            aux[:, 3 * bl + 0] = 1.0
            aux[:, 3 * bl + 1] = g_p[128 * gb:128 * gb + 128]
            aux[:, 3 * bl + 2] = g_n[128 * gb:128 * gb + 128]
        hs = slice(HSH * c, HSH * (c + 1))
        # w1: G-half (cov rows 4096..8191) first, then X-half
        w1rows = np.concatenate([fc_wT[1][DD:, hs], fc_wT[1][:DD, hs]], axis=0)
        w1 = w1rows.reshape(64, 128, HSH).transpose(1, 0, 2) \
            .reshape(128, 64 * HSH).astype(bf16)
        w2 = fc_wT[2][:, hs].reshape(KB, 128, HSH) \
            .transpose(1, 0, 2).reshape(128, KB * HSH).astype(bf16)
        w3 = fc_wT[3][:, hs].reshape(KB, 128, HSH) \
            .transpose(1, 0, 2).reshape(128, KB * HSH).astype(bf16)
        b1 = f64("fc1_b")[hs].reshape(4, 128).T.astype(np.float32)
        b2 = f64("fc2_b")[hs].reshape(4, 128).T.astype(np.float32)
        b3 = f64("fc3_b")[hs].reshape(1, HSH).astype(bf16)
        in_maps.append({
            "xt": xt, "xh": xh.astype(bf16), "tp": tp, "lm": lm,
            "aux": aux.astype(bf16), "w1": w1, "w2": w2, "w3": w3,
            "pj": pj, "b1": b1, "b2": b2, "b3": b3,
        })
    return in_maps


def run(inputs, trace=False, **kw):
    in_maps = prep_in_maps(inputs)
    if "nc" not in _CACHE:
        _CACHE["nc"] = build()
    nc = _CACHE["nc"]
    from concourse import bass_utils
    res = bass_utils.run_bass_kernel_spmd(nc, in_maps,
                                          core_ids=list(range(NCORES)),
                                          trace=trace, **kw)
    out = np.concatenate([res.results[c]["out"] for c in range(NCORES)], axis=1)
    return out.astype(np.float32), res


def kernel(**inputs) -> np.ndarray:
    out, _ = run(inputs)
    return out


if __name__ == "__main__":
    data = np.load("inputs.npz")
    inputs = {k: data[k] for k in data.files}
    expected = np.load("expected.npy")
    out = kernel(**inputs)
    scale = np.abs(expected).max()
    err = np.abs(out - expected).max() / scale
    print(f"Relative error: {err:.3e}")
